# revision 1
# baseline (speedup 1.0000x reference)
"""Multi-head causal attention (B=2, T=2048, C=1024, H=16, S=64) on 8 TRN2 cores.

Sharding: core i handles batch b = i//4 and head group g = i%4 (4 heads each).
Each core computes a partial output projection (its heads' contribution to the
full [T, C] output); the host sums the 4 partials per batch and adds the bias.

Per-core dataflow (all layouts chosen so no on-chip transposes are needed;
bf16 matmuls with f32 PSUM accumulation throughout):
  qT/kT   [S, T]  = W.T @ x.T        (head-pair packed in the 128 partitions)
  v       [T, S]                     (bf16 stored, ones column appended for d)
  attT    [Tk, Tq] = kT-tile.T @ qT  (K=64; exact-causal tiles only)
  p       = exp(0.125 * attT)        (ACT, bf16 out; diagonal masked via 0/1 mul)
  yT|d    [S+1, Tq] = [v|1].T @ p    (row 64 = softmax denominator)
  yT_norm = yT * (1/d)               (reciprocal + partition_broadcast + mul)
  out     [T, C] partial = yT.T @ WpT (head-pair stacked contraction)
"""

import os
import math
import numpy as np
import ml_dtypes

import concourse.bacc as bacc
import concourse.mybir as mybir
import concourse.tile as tile
from concourse.bass_utils import run_bass_kernel_spmd

F32 = mybir.dt.float32
F32R = mybir.dt.float32r
BF16 = mybir.dt.bfloat16

B, T, C, H, S = 2, 2048, 1024, 16, 64
HPC = 4          # heads per core
N_CORES = 8
NC_T = T // 128  # 16 t-tiles of 128

# attT storage offsets: tile tk spans tq in [128*tk, 2048)
SPAN = [T - 128 * tk for tk in range(NC_T)]
OFF = [0] * NC_T
for _tk in range(1, NC_T):
    OFF[_tk] = OFF[_tk - 1] + SPAN[_tk - 1]
ATT_W = OFF[-1] + SPAN[-1]  # 17408

_cached_nc = None
last_results = None  # BassKernelResults of the most recent run (for test harness)


def _build():
    nc = bacc.Bacc("TRN2", target_bir_lowering=False)

    # bf16 QKV inputs, pre-chunked on host so each DMA is one big contiguous-
    # per-partition transfer (128 rows x 2-8KB): c-chunk c of wq[hp] lives at
    # cols [128c:128c+128], of wv at cols [256c:256c+256].
    xT_d = nc.dram_tensor("xT", [C, T], BF16, kind="ExternalInput")
    wq_d = nc.dram_tensor("wq", [2, 128, 8 * 128], BF16, kind="ExternalInput")
    wk_d = nc.dram_tensor("wk", [2, 128, 8 * 128], BF16, kind="ExternalInput")
    wv_d = nc.dram_tensor("wv", [128, 8 * 256], BF16, kind="ExternalInput")
    wpT_d = nc.dram_tensor("wpT", [2, 128, C], BF16, kind="ExternalInput")
    mask_d = nc.dram_tensor("mask", [128, 128], BF16, kind="ExternalInput")
    out_d = nc.dram_tensor("out", [T, C], BF16, kind="ExternalOutput")

    with tile.TileContext(nc) as tc:
        with (
            tc.tile_pool(name="const", bufs=1) as constp,
            tc.tile_pool(name="qkT", bufs=1) as qkp,
            tc.tile_pool(name="vsb", bufs=1) as vp,
            tc.tile_pool(name="yT", bufs=1) as ytp,
            tc.tile_pool(name="attT", bufs=1) as attp,
            tc.tile_pool(name="yps", bufs=2, space="PSUM") as yps,
            tc.tile_pool(name="sm", bufs=2) as smp,
        ):
            # persistent tiles
            mask_sb = constp.tile([128, 128], BF16, name="mask_sb")
            nc.sync.dma_start(mask_sb[:], mask_d[:])

            qT2 = [qkp.tile([128, T], BF16, name=f"qT2_{hp}") for hp in range(2)]
            kT2 = [qkp.tile([128, T], BF16, name=f"kT2_{hp}") for hp in range(2)]
            # v tiles: [128, 4*65] bf16; head h in cols 65h..65h+63, col 65h+64 = 1
            v_sb = [vp.tile([128, 4 * 65], BF16, name=f"v{t}") for t in range(NC_T)]
            for t in range(NC_T):
                ones_ap = v_sb[t].rearrange("p (h c) -> p h c", h=4)[:, :, 64]
                nc.vector.memset(ones_ap, 1.0)
            yT_all = [ytp.tile([128, T], BF16, name=f"yTa{hp}") for hp in range(2)]
            att_buf = [
                attp.tile([128, ATT_W], BF16, name=f"attb{i}") for i in range(3)
            ]
            BUF_OF = [0, 1, 2, 0]  # head -> attT buffer

            def emit_scores_tk(h, tk):
                hp, half = h // 2, h % 2
                r0 = 64 * half
                ab = att_buf[BUF_OF[h]]
                krow = kT2[hp][r0 : r0 + 64, :]
                qrow = qT2[hp][r0 : r0 + 64, :]
                span = SPAN[tk]
                kt = krow[:, 128 * tk : 128 * tk + 128]
                for part in range(math.ceil(span / 1024)):
                    pspan = min(1024, span - 1024 * part)
                    pt = sps.tile([128, 1024], F32, name="sps_t", tag="s")
                    for mmi in range(math.ceil(pspan / 512)):
                        n = min(512, pspan - 512 * mmi)
                        tq0 = 128 * tk + 1024 * part + 512 * mmi
                        nc.tensor.matmul(
                            pt[:, 512 * mmi : 512 * mmi + n],
                            kt,
                            qrow[:, tq0 : tq0 + n],
                            start=True,
                            stop=True,
                        )
                    dst = ab[
                        :, OFF[tk] + 1024 * part : OFF[tk] + 1024 * part + pspan
                    ]
                    nc.scalar.activation(
                        dst,
                        pt[:, 0:pspan],
                        mybir.ActivationFunctionType.Exp,
                        scale=0.125,
                    )
                # mask the diagonal block (first 128 cols of this tk tile)
                diag = ab[:, OFF[tk] : OFF[tk] + 128]
                nc.vector.tensor_mul(diag, diag, mask_sb[:])

            def emit_y_window(h, j):
                hp, half = h // 2, h % 2
                ab = att_buf[BUF_OF[h]]
                yp = yps.tile([65, 512], F32, name="yps_t", tag="y")
                tk_hi = min(NC_T - 1, 4 * j + 3)
                for tk in range(tk_hi + 1):
                    if 128 * tk <= 512 * j:
                        n = 512
                        outc = 0
                        ac = OFF[tk] + 512 * j - 128 * tk
                    else:
                        n = 512 * (j + 1) - 128 * tk
                        outc = 128 * tk - 512 * j
                        ac = OFF[tk]
                    nc.tensor.matmul(
                        yp[:, outc : outc + n],
                        v_sb[tk][:, 65 * h : 65 * h + 65],
                        ab[:, ac : ac + n],
                        start=(tk == 0),
                        stop=(tk == tk_hi),
                        skip_group_check=True,
                    )
                # normalize: yT_norm = yT * (1/d), d in psum row 64
                rec = smp.tile([1, 512], F32, name="rec")
                nc.vector.reciprocal(rec[:], yp[64:65, :])
                bc = smp.tile([64, 512], F32, name="bc")
                nc.gpsimd.partition_broadcast(bc[:], rec[:])
                dst = yT_all[hp][
                    64 * half : 64 * half + 64, 512 * j : 512 * j + 512
                ]
                if half == 0:
                    nc.vector.tensor_mul(dst, yp[0:64, :], bc[:])
                else:
                    stg = smp.tile([64, 512], BF16, name="stg")
                    nc.vector.tensor_mul(stg[:], yp[0:64, :], bc[:])
                    # SWDGE queue: keeps the partition shift off the HWDGE
                    # queue that carries the big input/output transfers.
                    nc.gpsimd.dma_start(dst, stg[:])

            # ---- scores/QKV scope: sps closes after phase E ----
            wpT_sb = [
                constp.tile([128, C], BF16, name=f"wpT{hp}") for hp in range(2)
            ]
            with (
                tc.tile_pool(name="sps", bufs=2, space="PSUM") as sps,
            ):
              with (
                tc.tile_pool(name="xw", bufs=1) as xw,
                tc.tile_pool(name="mmps", bufs=2, space="PSUM") as mmps,
              ):
                # x first (the QK c-loop consumes chunks in order), weights
                # adjacent to first use; all transfers are 128 x 2-8KB rows.
                wq_sb = [
                    xw.tile([128, 1024], BF16, name=f"wq{hp}") for hp in range(2)
                ]
                wk_sb = [
                    xw.tile([128, 1024], BF16, name=f"wk{hp}") for hp in range(2)
                ]
                wv_sb = xw.tile([128, 2048], BF16, name="wv")
                xT_sb = [xw.tile([128, T], BF16, name=f"xT{c}") for c in range(8)]
                nc.sync.dma_start(wq_sb[0][:], wq_d[0])
                # half-major loads: the first two QK groups only need
                # cols 0-1023 of every chunk, so they can start after ~2MB
                # of the 4MB x transfer instead of all of it.
                for half in range(2):
                    for c in range(8):
                        nc.sync.dma_start(
                            xT_sb[c][:, 1024 * half : 1024 * half + 1024],
                            xT_d[
                                128 * c : 128 * c + 128,
                                1024 * half : 1024 * half + 1024,
                            ],
                        )
                nc.sync.dma_start(wk_sb[0][:], wk_d[0])
                nc.sync.dma_start(wv_sb[:], wv_d[:])
                nc.sync.dma_start(wq_sb[1][:], wq_d[1])
                nc.sync.dma_start(wk_sb[1][:], wk_d[1])

                def emit_qk_group(hp, kind, tq):
                    w_sb = wq_sb if kind == 0 else wk_sb
                    dst = qT2[hp] if kind == 0 else kT2[hp]
                    pt = mmps.tile([128, 512], F32, name="qkps", tag="qk")
                    for c in range(8):
                        nc.tensor.matmul(
                            pt[:],
                            w_sb[hp][:, 128 * c : 128 * c + 128],
                            xT_sb[c][:, 512 * tq : 512 * tq + 512],
                            start=(c == 0),
                            stop=(c == 7),
                        )
                    nc.vector.tensor_copy(dst[:, 512 * tq : 512 * tq + 512], pt[:])

                def emit_v_t(t):
                    pv = mmps.tile([128, 256], F32, name="vps", tag="qk")
                    for c in range(8):
                        nc.tensor.matmul(
                            pv[:],
                            xT_sb[c][:, 128 * t : 128 * t + 128],
                            wv_sb[:, 256 * c : 256 * c + 256],
                            start=(c == 0),
                            stop=(c == 7),
                        )
                    nc.vector.tensor_copy(
                        v_sb[t].rearrange("p (h c) -> p h c", h=4)[:, :, 0:64],
                        pv[:].rearrange("p (h c) -> p h c", h=4),
                    )

                # PE warm-up: dummy matmuls on the mask tile while the
                # first input DMAs are in flight (HAM clock-gate warm-up).
                warm = sps.tile([128, 1024], F32, name="warm", tag="s")
                for i in range(24):
                    nc.tensor.matmul(
                        warm[:, 0:128],
                        mask_sb[:],
                        mask_sb[:],
                        start=True,
                        stop=True,
                    )
                # Phase A: q projections for head-pair 0.
                for tq in range(4):
                    emit_qk_group(0, 0, tq)
                for hp in range(2):
                    nc.gpsimd.dma_start(wpT_sb[hp][:], wpT_d[hp])
                # Phase B: k(hp0) + scores h0 + q(hp1) filler.
                for g in range(4):
                    emit_qk_group(0, 1, g)
                    for tk in range(4 * g, 4 * g + 4):
                        emit_scores_tk(0, tk)
                    emit_qk_group(1, 0, g)
                # Phase C: k(hp1) + scores h1 + first half of v.
                for g in range(4):
                    emit_qk_group(1, 1, g)
                    for tk in range(4 * g, 4 * g + 4):
                        emit_scores_tk(1, tk)
                    emit_v_t(2 * g)
                    emit_v_t(2 * g + 1)
                # Phase D: scores h2 + second half of v + y(h0) windows.
                for g in range(4):
                    for tk in range(4 * g, 4 * g + 4):
                        emit_scores_tk(2, tk)
                    emit_v_t(8 + 2 * g)
                    emit_v_t(9 + 2 * g)
                    emit_y_window(0, g)

              # Phase E: scores h3 + y(h1) + y(h2) windows (sps still open).
              for g in range(4):
                  for tk in range(4 * g, 4 * g + 4):
                      emit_scores_tk(3, tk)
                  emit_y_window(1, g)
                  emit_y_window(2, g)

            # ---- projection (sps closed: pps gets its 4 banks) ----
            with (
                tc.tile_pool(name="pps", bufs=4, space="PSUM") as pps,
                tc.tile_pool(name="outs", bufs=8) as outs,
            ):
                def emit_proj_pair(t0):
                    # hp0 halves first: they depend only on earlier heads, so
                    # they hide the y(h3) normalize chain of the current batch.
                    pps_t = {}
                    for t in (t0, t0 + 1):
                        for n in range(2):
                            pp = pps.tile([128, 512], F32, name="pp", tag="p")
                            pps_t[t, n] = pp
                            nc.tensor.matmul(
                                pp[:],
                                yT_all[0][:, 128 * t : 128 * t + 128],
                                wpT_sb[0][:, 512 * n : 512 * n + 512],
                                start=True,
                                stop=False,
                                skip_group_check=True,
                            )
                    for t in (t0, t0 + 1):
                        for n in range(2):
                            pp = pps_t[t, n]
                            nc.tensor.matmul(
                                pp[:],
                                yT_all[1][:, 128 * t : 128 * t + 128],
                                wpT_sb[1][:, 512 * n : 512 * n + 512],
                                start=False,
                                stop=True,
                                skip_group_check=True,
                            )
                            ot = outs.tile([128, 512], BF16, name="ot")
                            # alternate engines: ACT is idle once exp is done
                            if n == 0:
                                nc.vector.tensor_copy(ot[:], pp[:])
                            else:
                                nc.scalar.copy(ot[:], pp[:])
                            # final batch: split across both DMA queues
                            eng = nc.gpsimd if (t >= 14 and n == 1) else nc.sync
                            eng.dma_start(
                                out_d[
                                    128 * t : 128 * t + 128,
                                    512 * n : 512 * n + 512,
                                ],
                                ot[:],
                            )

                # Phase F: y(h3) windows one batch ahead of their
                # projection, so each normalize chain hides under the
                # previous batch's proj matmuls.
                emit_y_window(3, 0)
                emit_y_window(3, 1)
                for j in range(4):
                    emit_proj_pair(4 * j)
                    if j < 2:
                        emit_y_window(3, j + 2)
                    emit_proj_pair(4 * j + 2)

    nc.finalize()
    return nc


def _get_nc():
    global _cached_nc
    if _cached_nc is None:
        _cached_nc = _build()
    return _cached_nc


def kernel(x, Wq, Wk, Wv, Wp, bp):
    global last_results
    x = np.asarray(x, dtype=np.float32)
    Wq = np.asarray(Wq, dtype=np.float32)
    Wk = np.asarray(Wk, dtype=np.float32)
    Wv = np.asarray(Wv, dtype=np.float32)
    Wp = np.asarray(Wp, dtype=np.float32)
    bp = np.asarray(bp, dtype=np.float32)

    WpT = np.ascontiguousarray(Wp.T)  # [C_in(features), C_out]
    mask01 = np.triu(np.ones((128, 128), dtype=np.float32)).astype(ml_dtypes.bfloat16)

    def chunked(w):
        # [C, m] -> [128, 8*m]: c-chunk c at cols [m*c : m*(c+1)]
        m = w.shape[1]
        return np.ascontiguousarray(
            w.reshape(8, 128, m).transpose(1, 0, 2).reshape(128, 8 * m)
        ).astype(ml_dtypes.bfloat16)

    xT_by_batch = [
        np.ascontiguousarray(x[b].T).astype(ml_dtypes.bfloat16) for b in range(B)
    ]
    in_maps = []
    for core in range(N_CORES):
        b, g = core // 4, core % 4
        h0 = HPC * g
        wq_p = np.stack(
            [chunked(np.concatenate([Wq[h0 + 2 * hp], Wq[h0 + 2 * hp + 1]], axis=1))
             for hp in range(2)]
        )  # [2, 128, 1024] bf16
        wk_p = np.stack(
            [chunked(np.concatenate([Wk[h0 + 2 * hp], Wk[h0 + 2 * hp + 1]], axis=1))
             for hp in range(2)]
        )
        wv_p = chunked(
            np.concatenate([Wv[h0 + j] for j in range(HPC)], axis=1)
        )  # [128, 2048] bf16
        wpT_p = np.ascontiguousarray(
            WpT[256 * g : 256 * (g + 1)].reshape(2, 128, C)
        ).astype(ml_dtypes.bfloat16)
        in_maps.append(
            {
                "xT": xT_by_batch[b],
                "wq": wq_p,
                "wk": wk_p,
                "wv": wv_p,
                "wpT": wpT_p,
                "mask": mask01,
            }
        )

    nc = _get_nc()
    kwargs = {}
    if os.environ.get("KERNEL_TRACE", "0") == "1":
        kwargs = dict(trace=True, trace_cores=list(range(N_CORES)),
                      stitch_traces=True)
    try:
        res = run_bass_kernel_spmd(
            nc, in_maps, core_ids=list(range(N_CORES)), **kwargs
        )
    except ModuleNotFoundError:
        # tracing unavailable in this environment; run untraced
        res = run_bass_kernel_spmd(nc, in_maps, core_ids=list(range(N_CORES)))
    last_results = res

    out = np.zeros((B, T, C), dtype=np.float32)
    for core in range(N_CORES):
        b = core // 4
        out[b] += res.results[core]["out"].astype(np.float32)
    out += bp[None, None, :]
    return out



# revision 22
# speedup vs baseline: 1.0056x; 1.0056x over previous
"""Multi-head causal attention (B=2, T=2048, C=1024, H=16, S=64) on 8 TRN2 cores.

Sharding: core i handles batch b = i//4 and head group g = i%4 (4 heads each).
Each core computes a partial output projection (its heads' contribution to the
full [T, C] output); the host sums the 4 partials per batch and adds the bias.

Per-core dataflow (f32 PSUM accumulation throughout):
  qT/kT  [128, T] bf16 = (32W).T @ x   -- fp8 DoubleRow over c-chunk pairs,
         3 passes (hi*hi + hi*Wlo + xlo*hi) so fp8 quantization cancels
  v      [128, 4h x 65] bf16 (32-scaled, ones column for the denominator),
         same 3-pass fp8 DR projection
  attT   [tk, tq] psum = kT-tile.T @ qT (bf16, K=64) with a -1e6 upper-
         triangle matmul accumulated onto the diagonal block (causal mask)
  p      = exp(att/(sqrt(S)*1024))     (ACT, bf16 out; mask underflows to 0)
  y      [tq=128, 65] psum per t-tile = p-tile.T @ [v|1]  (flipped: N=65)
  ynorm  = y * recip(d) per-partition (DVE tensor_scalar), staged per
         head-pair then PE-transposed back to yT [s, t] layout
  out    [T, C] partial = yT.T @ (WpT/32) (bf16, head-pair accumulation)

Heads processed in two phases (pair 0-1 then 2-3) so only two bf16 attT
buffers are live; the projection runs in phase 2, gated per t-tile.
"""

import os
import math
import numpy as np
import ml_dtypes

import concourse.bacc as bacc
import concourse.mybir as mybir
import concourse.tile as tile
from concourse.bass_utils import run_bass_kernel_spmd

F32 = mybir.dt.float32
BF16 = mybir.dt.bfloat16
FP8 = mybir.dt.float8e4
DR = mybir.MatmulPerfMode.DoubleRow

B, T, C, H, S = 2, 2048, 1024, 16, 64
HPC = 4          # heads per core
N_CORES = 8
NC_T = T // 128  # 16 t-tiles of 128

WSCALE = 32.0                       # weight scale (fp8 denormal avoidance)
ESCALE = 0.125 / (WSCALE * WSCALE)  # exp scale on (32q)(32k) psum logits
BIGL = 1.0e6                        # causal-mask logit offset (pre-ESCALE)

# attT storage offsets: tile tk spans tq in [128*tk, 2048)
SPAN = [T - 128 * tk for tk in range(NC_T)]
OFF = [0] * NC_T
for _tk in range(1, NC_T):
    OFF[_tk] = OFF[_tk - 1] + SPAN[_tk - 1]
ATT_W = OFF[-1] + SPAN[-1]  # 17408

_cached_nc = None
last_results = None  # BassKernelResults of the most recent run (for test harness)


def _build():
    nc = bacc.Bacc("TRN2", target_bir_lowering=False)

    # contraction row c = 128*s + p lives at partition p, slot s.
    # *l tensors are the fp8 quantization residuals.
    xT_d = nc.dram_tensor("xT", [128, 8 * T], FP8, kind="ExternalInput")
    xl_d = nc.dram_tensor("xl", [128, 8 * T], FP8, kind="ExternalInput")
    wq_d = nc.dram_tensor("wq", [2, 128, 8 * 128], FP8, kind="ExternalInput")
    wk_d = nc.dram_tensor("wk", [2, 128, 8 * 128], FP8, kind="ExternalInput")
    wql_d = nc.dram_tensor("wql", [2, 128, 8 * 128], FP8, kind="ExternalInput")
    wkl_d = nc.dram_tensor("wkl", [2, 128, 8 * 128], FP8, kind="ExternalInput")
    wv_d = nc.dram_tensor("wv", [128, 8 * 256], FP8, kind="ExternalInput")
    wvl_d = nc.dram_tensor("wvl", [128, 8 * 256], FP8, kind="ExternalInput")
    wpT_d = nc.dram_tensor("wpT", [2, 128, C], BF16, kind="ExternalInput")
    ub_d = nc.dram_tensor("ub", [128, 128], BF16, kind="ExternalInput")
    id_d = nc.dram_tensor("id", [128, 128], BF16, kind="ExternalInput")
    out_d = nc.dram_tensor("out", [T, C], BF16, kind="ExternalOutput")

    with tile.TileContext(nc) as tc:
        with (
            tc.tile_pool(name="const", bufs=1) as constp,
            tc.tile_pool(name="qkT", bufs=1) as qkp,
            tc.tile_pool(name="vsb", bufs=1) as vp,
            tc.tile_pool(name="yT", bufs=1) as ytp,
            tc.tile_pool(name="attT", bufs=1) as attp,
            tc.tile_pool(name="sps", bufs=2, space="PSUM") as sps,
            tc.tile_pool(name="mps", bufs=2, space="PSUM") as mps,
            tc.tile_pool(name="yps", bufs=2, space="PSUM") as yps,
            tc.tile_pool(name="xw", bufs=1) as xw,
            tc.tile_pool(name="outs", bufs=4) as outs,
            tc.tile_pool(name="sm", bufs=5) as smp,
        ):
            ub_sb = constp.tile([128, 128], BF16, name="ub_sb")
            id_sb = constp.tile([128, 128], BF16, name="id_sb")
            nc.gpsimd.dma_start(ub_sb[:], ub_d[:])
            nc.gpsimd.dma_start(id_sb[:], id_d[:])
            wpT_sb = [
                constp.tile([128, C], BF16, name=f"wpT{hp}") for hp in range(2)
            ]

            qT2 = [qkp.tile([128, T], BF16, name=f"qT2_{hp}") for hp in range(2)]
            kT2 = [qkp.tile([128, T], BF16, name=f"kT2_{hp}") for hp in range(2)]
            # v tiles: [128, 4 heads x 65] bf16; col 64 of each 65-block = 1
            v_sb = [vp.tile([128, 4 * 65], BF16, name=f"v{t}") for t in range(NC_T)]
            for t in range(NC_T):
                nc.vector.memset(
                    v_sb[t].rearrange("p (h c) -> p h c", h=4)[:, :, 64], 1.0
                )
            yT_all = [ytp.tile([128, T], BF16, name=f"yTa{hp}") for hp in range(2)]
            # three attT buffers: h0->0, h1->1, h2->2, h3->0. The third
            # lets phase B's first head start before phase A's y tail drains.
            ab = [attp.tile([128, ATT_W], BF16, name=f"attb{i}") for i in range(3)]
            ABMAP = {0: 0, 1: 1, 2: 2, 3: 0}

            # input DMAs: x8 pairs + hp0 hi weights first, then residuals
            wq_sb = [xw.tile([128, 1024], FP8, name=f"wq{hp}") for hp in range(2)]
            wk_sb = [xw.tile([128, 1024], FP8, name=f"wk{hp}") for hp in range(2)]
            wql_sb = [xw.tile([128, 1024], FP8, name=f"wql{hp}") for hp in range(2)]
            wkl_sb = [xw.tile([128, 1024], FP8, name=f"wkl{hp}") for hp in range(2)]
            wv_sb = xw.tile([128, 2048], FP8, name="wv")
            wvl_sb = xw.tile([128, 2048], FP8, name="wvl")
            xT_sb = xw.tile([128, 8 * T], FP8, name="xT")
            xl_sb = xw.tile([128, 8 * T], FP8, name="xl")

            nc.sync.dma_start(xT_sb[:, 0:4096], xT_d[:, 0:4096])
            nc.sync.dma_start(wq_sb[0][:], wq_d[0])
            nc.sync.dma_start(wk_sb[0][:], wk_d[0])
            for sp in range(1, 4):
                nc.sync.dma_start(
                    xT_sb[:, 4096 * sp : 4096 * sp + 4096],
                    xT_d[:, 4096 * sp : 4096 * sp + 4096],
                )
            nc.sync.dma_start(wql_sb[0][:], wql_d[0])
            nc.sync.dma_start(wkl_sb[0][:], wkl_d[0])
            for sp in range(4):
                nc.sync.dma_start(
                    xl_sb[:, 4096 * sp : 4096 * sp + 4096],
                    xl_d[:, 4096 * sp : 4096 * sp + 4096],
                )
            nc.sync.dma_start(wv_sb[:], wv_d[:])
            nc.sync.dma_start(wvl_sb[:], wvl_d[:])
            nc.sync.dma_start(wq_sb[1][:], wq_d[1])
            nc.sync.dma_start(wk_sb[1][:], wk_d[1])
            nc.sync.dma_start(wql_sb[1][:], wql_d[1])
            nc.sync.dma_start(wkl_sb[1][:], wkl_d[1])
            for hp in range(2):
                nc.gpsimd.dma_start(wpT_sb[hp][:], wpT_d[hp])

            x_sl = xT_sb.rearrange("p (s t) -> p s t", s=8)
            xl_sl = xl_sb.rearrange("p (s t) -> p s t", s=8)

            def emit_qk_group(hp, kind, tq):
                w_hi = (wq_sb if kind == 0 else wk_sb)[hp]
                w_lo = (wql_sb if kind == 0 else wkl_sb)[hp]
                dst = (qT2 if kind == 0 else kT2)[hp]
                pt = mps.tile([128, 512], F32, name="mp_t", tag="mp")
                passes = ((w_hi, x_sl), (w_lo, x_sl), (w_hi, xl_sl))
                for pi, (wgt, xs) in enumerate(passes):
                    for jj in range(4):
                        lhsT = wgt[:, 256 * jj : 256 * jj + 256].rearrange(
                            "p (j m) -> p j m", j=2
                        )
                        rhs = xs[:, 2 * jj : 2 * jj + 2, 512 * tq : 512 * tq + 512]
                        nc.tensor.matmul(
                            pt[:],
                            lhsT,
                            rhs,
                            start=(pi == 0 and jj == 0),
                            stop=(pi == 2 and jj == 3),
                            perf_mode=DR,
                            skip_group_check=True,
                        )
                # hp0 q copies on ACT (idle pre-stream); rest on DVE
                if kind == 0 and hp == 0:
                    nc.scalar.copy(dst[:, 512 * tq : 512 * tq + 512], pt[:])
                else:
                    nc.vector.tensor_copy(dst[:, 512 * tq : 512 * tq + 512], pt[:])

            def emit_v_t(t):
                pv = mps.tile([128, 512], F32, name="mp_t", tag="mp")
                passes = ((wv_sb, x_sl), (wvl_sb, x_sl), (wv_sb, xl_sl))
                for pi, (wgt, xs) in enumerate(passes):
                    for jj in range(4):
                        lhsT = xs[:, 2 * jj : 2 * jj + 2, 128 * t : 128 * t + 128]
                        rhs = wgt[:, 512 * jj : 512 * jj + 512].rearrange(
                            "p (j m) -> p j m", j=2
                        )
                        nc.tensor.matmul(
                            pv[:, 0:256],
                            lhsT,
                            rhs,
                            start=(pi == 0 and jj == 0),
                            stop=(pi == 2 and jj == 3),
                            perf_mode=DR,
                            skip_group_check=True,
                        )
                dst = v_sb[t].rearrange("p (h c) -> p h c", h=4)[:, :, 0:64]
                nc.vector.tensor_copy(
                    dst, pv[:, 0:256].rearrange("p (h c) -> p h c", h=4)
                )

            def emit_scores_tk(h, tk):
                # h is the core-local head index 0..3; buffer = h % 2
                hp, half = h // 2, h % 2
                r0 = 64 * half
                krow = kT2[hp][r0 : r0 + 64, :]
                qrow = qT2[hp][r0 : r0 + 64, :]
                span = SPAN[tk]
                kt = krow[:, 128 * tk : 128 * tk + 128]
                for part in range(math.ceil(span / 1024)):
                    pspan = min(1024, span - 1024 * part)
                    pt = sps.tile([128, 1024], F32, name="sps_t", tag="s")
                    for mmi in range(math.ceil(pspan / 512)):
                        n = min(512, pspan - 512 * mmi)
                        tq0 = 128 * tk + 1024 * part + 512 * mmi
                        nc.tensor.matmul(
                            pt[:, 512 * mmi : 512 * mmi + n],
                            kt,
                            qrow[:, tq0 : tq0 + n],
                            start=True,
                            stop=(part > 0 or mmi > 0),
                            skip_group_check=True,
                        )
                        if part == 0 and mmi == 0:
                            # causal mask: accumulate -BIGL above the diagonal
                            nc.tensor.matmul(
                                pt[:, 0:128],
                                ub_sb[:],
                                id_sb[:],
                                start=False,
                                stop=True,
                                skip_group_check=True,
                            )
                    dst = ab[ABMAP[h]][
                        :, OFF[tk] + 1024 * part : OFF[tk] + 1024 * part + pspan
                    ]
                    nc.scalar.activation(
                        dst,
                        pt[:, 0:pspan],
                        mybir.ActivationFunctionType.Exp,
                        scale=ESCALE,
                    )

            def emit_y_t(h, t, st):
                """Flipped y for tq-tile t: yp[tq, s|d] = sum_tk p.T @ [v|1];
                normalize into staging tile st cols [64*(h%2) : +64]."""
                buf = ab[ABMAP[h]][:]
                yp = yps.tile([128, 65], F32, name="yps_t", tag="y")
                for tk in range(t + 1):
                    c = OFF[tk] + 128 * (t - tk)
                    nc.tensor.matmul(
                        yp[:],
                        buf[:, c : c + 128],
                        v_sb[tk].rearrange("p (h c) -> p h c", h=4)[:, h],
                        start=(tk == 0),
                        stop=(tk == t),
                        skip_group_check=True,
                    )
                rec = smp.tile([128, 1], F32, name="rec")
                nc.vector.reciprocal(rec[:], yp[:, 64:65])
                nc.vector.tensor_scalar_mul(
                    st[:, 64 * (h % 2) : 64 * (h % 2) + 64], yp[:, 0:64], rec[:]
                )

            def emit_y_pair(hp, t):
                """y for both heads of the pair into a staging tile."""
                st = smp.tile([128, 128], BF16, name="st")
                for h in (2 * hp, 2 * hp + 1):
                    emit_y_t(h, t, st)
                return st

            def emit_yt_transpose(hp, t, st):
                tp = mps.tile([128, 128], BF16, name="mp_t", tag="mp")
                nc.tensor.transpose(tp[:], st[:], id_sb[:])
                nc.vector.tensor_copy(yT_all[hp][:, 128 * t : 128 * t + 128], tp[:])

            def emit_proj_t(t, late):
                pps_t = {}
                for n in range(2):
                    pp = mps.tile([128, 512], F32, name="mp_t", tag="mp")
                    pps_t[n] = pp
                    nc.tensor.matmul(
                        pp[:],
                        yT_all[0][:, 128 * t : 128 * t + 128],
                        wpT_sb[0][:, 512 * n : 512 * n + 512],
                        start=True,
                        stop=False,
                        skip_group_check=True,
                    )
                ot = outs.tile([128, 1024], BF16, name="ot")
                for n in range(2):
                    pp = pps_t[n]
                    nc.tensor.matmul(
                        pp[:],
                        yT_all[1][:, 128 * t : 128 * t + 128],
                        wpT_sb[1][:, 512 * n : 512 * n + 512],
                        start=False,
                        stop=True,
                        skip_group_check=True,
                    )
                    # tail tiles: ACT is free once the exp stream ends
                    if late and n == 1:
                        nc.scalar.copy(ot[:, 512 * n : 512 * n + 512], pp[:])
                    else:
                        nc.vector.tensor_copy(
                            ot[:, 512 * n : 512 * n + 512], pp[:]
                        )
                eng = nc.gpsimd if t >= 14 else nc.sync
                eng.dma_start(out_d[128 * t : 128 * t + 128, :], ot[:])

            # PE warm-up on the ub tile while input DMAs land
            warm = sps.tile([128, 1024], F32, name="warm", tag="s")
            for i in range(24):
                nc.tensor.matmul(
                    warm[:, 0:128], ub_sb[:], ub_sb[:], start=True, stop=True
                )

            # ---- phase A: heads 0-1 ----
            for tq in range(4):
                emit_qk_group(0, 0, tq)
                emit_qk_group(0, 1, tq)
            # head-pair-1 QKV groups are drip-fed between score tiles of
            # rounds 2-3 so no single PE detour outruns the exp backlog
            qk1_queue = [(1, kind, tq) for tq in range(4) for kind in (0, 1)]
            for g in range(4):
                for h in (0, 1):
                    for tk in range(4 * g, 4 * g + 4):
                        emit_scores_tk(h, tk)
                        if h == 0:
                            emit_v_t(tk)
                        elif g >= 2 and qk1_queue:
                            emit_qk_group(*qk1_queue.pop(0))
                if g > 0:
                    sts = [emit_y_pair(0, t) for t in range(4 * (g - 1), 4 * g)]
                    for i, t in enumerate(range(4 * (g - 1), 4 * g)):
                        emit_yt_transpose(0, t, sts[i])
            sts = [emit_y_pair(0, t) for t in range(12, 16)]
            for i, t in enumerate(range(12, 16)):
                emit_yt_transpose(0, t, sts[i])

            # ---- phase B: heads 2-3 + projection ----
            for g in range(4):
                for h in (2, 3):
                    for tk in range(4 * g, 4 * g + 4):
                        emit_scores_tk(h, tk)
                if g > 0:
                    ts = list(range(4 * (g - 1), 4 * g))
                    sts = [emit_y_pair(1, t) for t in ts]
                    for i, t in enumerate(ts):
                        emit_yt_transpose(1, t, sts[i])
                    for t in ts:
                        emit_proj_t(t, late=False)
            ts = list(range(12, 16))
            sts = [emit_y_pair(1, t) for t in ts]
            for i, t in enumerate(ts):
                emit_yt_transpose(1, t, sts[i])
            for t in ts:
                emit_proj_t(t, late=True)

    nc.finalize()
    return nc


def _get_nc():
    global _cached_nc
    if _cached_nc is None:
        _cached_nc = _build()
    return _cached_nc


def kernel(x, Wq, Wk, Wv, Wp, bp):
    global last_results
    x = np.asarray(x, dtype=np.float32)
    Wq = np.asarray(Wq, dtype=np.float32)
    Wk = np.asarray(Wk, dtype=np.float32)
    Wv = np.asarray(Wv, dtype=np.float32)
    Wp = np.asarray(Wp, dtype=np.float32)
    bp = np.asarray(bp, dtype=np.float32)

    WpT = np.ascontiguousarray(Wp.T) / WSCALE  # [C_in(features), C_out]
    ub = np.triu(np.full((128, 128), -BIGL, dtype=np.float32), k=1).astype(
        ml_dtypes.bfloat16
    )
    id128 = np.eye(128, dtype=np.float32).astype(ml_dtypes.bfloat16)
    f8 = ml_dtypes.float8_e4m3

    def chunked(w):
        # [C, m] -> [128, 8*m]: c-chunk s at cols [m*s : m*(s+1)]
        m = w.shape[1]
        return np.ascontiguousarray(
            w.reshape(8, 128, m).transpose(1, 0, 2).reshape(128, 8 * m)
        )

    def hi_lo(w):
        hi = w.astype(f8)
        lo = (w - hi.astype(np.float32)).astype(f8)
        return hi, lo

    x8_by_b, xl_by_b = [], []
    for b in range(B):
        hi, lo = hi_lo(chunked(np.ascontiguousarray(x[b].T)))
        x8_by_b.append(hi)
        xl_by_b.append(lo)

    in_maps = []
    for core in range(N_CORES):
        b, g = core // 4, core % 4
        h0 = HPC * g
        wq_c = np.stack([chunked(np.concatenate(
            [Wq[h0 + 2 * hp] * WSCALE, Wq[h0 + 2 * hp + 1] * WSCALE], axis=1))
            for hp in range(2)])
        wk_c = np.stack([chunked(np.concatenate(
            [Wk[h0 + 2 * hp] * WSCALE, Wk[h0 + 2 * hp + 1] * WSCALE], axis=1))
            for hp in range(2)])
        wv_c = chunked(np.concatenate(
            [Wv[h0 + j] * WSCALE for j in range(HPC)], axis=1))
        wq_hi, wq_lo = hi_lo(wq_c)
        wk_hi, wk_lo = hi_lo(wk_c)
        wv_hi, wv_lo = hi_lo(wv_c)
        wpT_p = np.ascontiguousarray(
            WpT[256 * g : 256 * (g + 1)].reshape(2, 128, C)
        ).astype(ml_dtypes.bfloat16)
        in_maps.append({
            "xT": x8_by_b[b], "xl": xl_by_b[b],
            "wq": wq_hi, "wql": wq_lo,
            "wk": wk_hi, "wkl": wk_lo,
            "wv": wv_hi, "wvl": wv_lo,
            "wpT": wpT_p, "ub": ub, "id": id128,
        })

    nc = _get_nc()
    kwargs = {}
    if os.environ.get("KERNEL_TRACE", "0") == "1":
        kwargs = dict(trace=True, trace_cores=list(range(N_CORES)),
                      stitch_traces=True)
    try:
        res = run_bass_kernel_spmd(
            nc, in_maps, core_ids=list(range(N_CORES)), **kwargs
        )
    except ModuleNotFoundError:
        res = run_bass_kernel_spmd(nc, in_maps, core_ids=list(range(N_CORES)))
    last_results = res

    out = np.zeros((B, T, C), dtype=np.float32)
    for core in range(N_CORES):
        b = core // 4
        out[b] += res.results[core]["out"].astype(np.float32)
    out += bp[None, None, :]
    return out


# revision 25
# speedup vs baseline: 1.0164x; 1.0107x over previous
"""Multi-head causal attention (B=2, T=2048, C=1024, H=16, S=64) on 8 TRN2 cores.

Sharding: core i handles batch b = i//4 and head group g = i%4 (4 heads each).
Each core computes a partial output projection (its heads' contribution to the
full [T, C] output); the host sums the 4 partials per batch and adds the bias.

Per-core dataflow (f32 PSUM accumulation throughout):
  qT/kT  [128, T] bf16 = (32W).T @ x   -- fp8 DoubleRow over c-chunk pairs,
         3 passes (hi*hi + hi*Wlo + xlo*hi) so fp8 quantization cancels
  v      [128, 4h x 65] bf16 (32-scaled, ones column for the denominator),
         same 3-pass fp8 DR projection
  attT   [tk, tq] psum = kT-tile.T @ qT (bf16, K=64) with a -1e6 upper-
         triangle matmul accumulated onto the diagonal block (causal mask)
  p      = exp(att/(sqrt(S)*1024))     (ACT, bf16 out; mask underflows to 0)
  y      [tq=128, 65] psum per t-tile = p-tile.T @ [v|1]  (flipped: N=65)
  ynorm  = y * recip(d) per-partition (DVE tensor_scalar), staged per
         head-pair then PE-transposed back to yT [s, t] layout
  out    [T, C] partial = yT.T @ (WpT/32) (bf16, head-pair accumulation)

Heads processed in two phases (pair 0-1 then 2-3) so only two bf16 attT
buffers are live; the projection runs in phase 2, gated per t-tile.
"""

import os
import math
import numpy as np
import ml_dtypes

import concourse.bacc as bacc
import concourse.mybir as mybir
import concourse.tile as tile
from concourse.bass_utils import run_bass_kernel_spmd

F32 = mybir.dt.float32
BF16 = mybir.dt.bfloat16
FP8 = mybir.dt.float8e4
DR = mybir.MatmulPerfMode.DoubleRow

B, T, C, H, S = 2, 2048, 1024, 16, 64
HPC = 4          # heads per core
N_CORES = 8
NC_T = T // 128  # 16 t-tiles of 128

WSCALE = 32.0                       # weight scale (fp8 denormal avoidance)
ESCALE = 0.125 / (WSCALE * WSCALE)  # exp scale on (32q)(32k) psum logits
BIGL = 1.0e6                        # causal-mask logit offset (pre-ESCALE)

# attT storage offsets: tile tk spans tq in [128*tk, 2048)
SPAN = [T - 128 * tk for tk in range(NC_T)]
OFF = [0] * NC_T
for _tk in range(1, NC_T):
    OFF[_tk] = OFF[_tk - 1] + SPAN[_tk - 1]
ATT_W = OFF[-1] + SPAN[-1]  # 17408

_cached_nc = None
last_results = None  # BassKernelResults of the most recent run (for test harness)


def _build():
    nc = bacc.Bacc("TRN2", target_bir_lowering=False)

    # contraction row c = 128*s + p lives at partition p, slot s.
    # *l tensors are the fp8 quantization residuals.
    xT_d = nc.dram_tensor("xT", [128, 8 * T], FP8, kind="ExternalInput")
    xl_d = nc.dram_tensor("xl", [128, 8 * T], FP8, kind="ExternalInput")
    wq_d = nc.dram_tensor("wq", [2, 128, 8 * 128], FP8, kind="ExternalInput")
    wk_d = nc.dram_tensor("wk", [2, 128, 8 * 128], FP8, kind="ExternalInput")
    wql_d = nc.dram_tensor("wql", [2, 128, 8 * 128], FP8, kind="ExternalInput")
    wkl_d = nc.dram_tensor("wkl", [2, 128, 8 * 128], FP8, kind="ExternalInput")
    wv_d = nc.dram_tensor("wv", [128, 8 * 256], FP8, kind="ExternalInput")
    wvl_d = nc.dram_tensor("wvl", [128, 8 * 256], FP8, kind="ExternalInput")
    wpT_d = nc.dram_tensor("wpT", [2, 128, C], BF16, kind="ExternalInput")
    ub_d = nc.dram_tensor("ub", [128, 128], BF16, kind="ExternalInput")
    id_d = nc.dram_tensor("id", [128, 128], BF16, kind="ExternalInput")
    out_d = nc.dram_tensor("out", [T, C], BF16, kind="ExternalOutput")

    with tile.TileContext(nc) as tc:
        with (
            tc.tile_pool(name="const", bufs=1) as constp,
            tc.tile_pool(name="qkT", bufs=1) as qkp,
            tc.tile_pool(name="vsb", bufs=1) as vp,
            tc.tile_pool(name="yT", bufs=1) as ytp,
            tc.tile_pool(name="attT", bufs=1) as attp,
            tc.tile_pool(name="sps", bufs=2, space="PSUM") as sps,
            tc.tile_pool(name="mps", bufs=2, space="PSUM") as mps,
            tc.tile_pool(name="yps", bufs=2, space="PSUM") as yps,
            tc.tile_pool(name="xw", bufs=1) as xw,
            tc.tile_pool(name="outs", bufs=4) as outs,
            tc.tile_pool(name="sm", bufs=5) as smp,
        ):
            ub_sb = constp.tile([128, 128], BF16, name="ub_sb")
            id_sb = constp.tile([128, 128], BF16, name="id_sb")
            nc.gpsimd.dma_start(ub_sb[:], ub_d[:])
            nc.gpsimd.dma_start(id_sb[:], id_d[:])
            wpT_sb = [
                constp.tile([128, C], BF16, name=f"wpT{hp}") for hp in range(2)
            ]

            qT2 = [qkp.tile([128, T], BF16, name=f"qT2_{hp}") for hp in range(2)]
            kT2 = [qkp.tile([128, T], BF16, name=f"kT2_{hp}") for hp in range(2)]
            # v tiles: [128, 4 heads x 65] bf16; col 64 of each 65-block = 1
            v_sb = [vp.tile([128, 4 * 65], BF16, name=f"v{t}") for t in range(NC_T)]
            for t in range(NC_T):
                nc.vector.memset(
                    v_sb[t].rearrange("p (h c) -> p h c", h=4)[:, :, 64], 1.0
                )
            yT_all = [ytp.tile([128, T], BF16, name=f"yTa{hp}") for hp in range(2)]
            # three attT buffers: h0->0, h1->1, h2->2, h3->0. The third
            # lets phase B's first head start before phase A's y tail drains.
            ab = [attp.tile([128, ATT_W], BF16, name=f"attb{i}") for i in range(3)]
            ABMAP = {0: 0, 1: 1, 2: 2, 3: 0}

            # input DMAs: x8 pairs + hp0 hi weights first, then residuals
            wq_sb = [xw.tile([128, 1024], FP8, name=f"wq{hp}") for hp in range(2)]
            wk_sb = [xw.tile([128, 1024], FP8, name=f"wk{hp}") for hp in range(2)]
            wql_sb = [xw.tile([128, 1024], FP8, name=f"wql{hp}") for hp in range(2)]
            wkl_sb = [xw.tile([128, 1024], FP8, name=f"wkl{hp}") for hp in range(2)]
            wv_sb = xw.tile([128, 2048], FP8, name="wv")
            wvl_sb = xw.tile([128, 2048], FP8, name="wvl")
            xT_sb = xw.tile([128, 8 * T], FP8, name="xT")
            xl_sb = xw.tile([128, 8 * T], FP8, name="xl")

            nc.sync.dma_start(xT_sb[:, 0:4096], xT_d[:, 0:4096])
            nc.sync.dma_start(wq_sb[0][:], wq_d[0])
            nc.sync.dma_start(wk_sb[0][:], wk_d[0])
            for sp in range(1, 4):
                nc.sync.dma_start(
                    xT_sb[:, 4096 * sp : 4096 * sp + 4096],
                    xT_d[:, 4096 * sp : 4096 * sp + 4096],
                )
            nc.sync.dma_start(wql_sb[0][:], wql_d[0])
            nc.sync.dma_start(wkl_sb[0][:], wkl_d[0])
            for sp in range(4):
                nc.sync.dma_start(
                    xl_sb[:, 4096 * sp : 4096 * sp + 4096],
                    xl_d[:, 4096 * sp : 4096 * sp + 4096],
                )
            nc.sync.dma_start(wv_sb[:], wv_d[:])
            nc.sync.dma_start(wvl_sb[:], wvl_d[:])
            nc.sync.dma_start(wq_sb[1][:], wq_d[1])
            nc.sync.dma_start(wk_sb[1][:], wk_d[1])
            nc.sync.dma_start(wql_sb[1][:], wql_d[1])
            nc.sync.dma_start(wkl_sb[1][:], wkl_d[1])
            for hp in range(2):
                nc.gpsimd.dma_start(wpT_sb[hp][:], wpT_d[hp])

            x_sl = xT_sb.rearrange("p (s t) -> p s t", s=8)
            xl_sl = xl_sb.rearrange("p (s t) -> p s t", s=8)

            def emit_qk_group(hp, kind, tq):
                w_hi = (wq_sb if kind == 0 else wk_sb)[hp]
                w_lo = (wql_sb if kind == 0 else wkl_sb)[hp]
                dst = (qT2 if kind == 0 else kT2)[hp]
                pt = mps.tile([128, 512], F32, name="mp_t", tag="mp")
                passes = ((w_hi, x_sl), (w_lo, x_sl), (w_hi, xl_sl))
                for pi, (wgt, xs) in enumerate(passes):
                    for jj in range(4):
                        lhsT = wgt[:, 256 * jj : 256 * jj + 256].rearrange(
                            "p (j m) -> p j m", j=2
                        )
                        rhs = xs[:, 2 * jj : 2 * jj + 2, 512 * tq : 512 * tq + 512]
                        nc.tensor.matmul(
                            pt[:],
                            lhsT,
                            rhs,
                            start=(pi == 0 and jj == 0),
                            stop=(pi == 2 and jj == 3),
                            perf_mode=DR,
                            skip_group_check=True,
                        )
                # hp0 q copies on ACT (idle pre-stream); rest on DVE
                if kind == 0 and hp == 0:
                    nc.scalar.copy(dst[:, 512 * tq : 512 * tq + 512], pt[:])
                else:
                    nc.vector.tensor_copy(dst[:, 512 * tq : 512 * tq + 512], pt[:])

            def emit_v_t(t):
                pv = mps.tile([128, 512], F32, name="mp_t", tag="mp")
                passes = ((wv_sb, x_sl), (wvl_sb, x_sl), (wv_sb, xl_sl))
                for pi, (wgt, xs) in enumerate(passes):
                    for jj in range(4):
                        lhsT = xs[:, 2 * jj : 2 * jj + 2, 128 * t : 128 * t + 128]
                        rhs = wgt[:, 512 * jj : 512 * jj + 512].rearrange(
                            "p (j m) -> p j m", j=2
                        )
                        nc.tensor.matmul(
                            pv[:, 0:256],
                            lhsT,
                            rhs,
                            start=(pi == 0 and jj == 0),
                            stop=(pi == 2 and jj == 3),
                            perf_mode=DR,
                            skip_group_check=True,
                        )
                dst = v_sb[t].rearrange("p (h c) -> p h c", h=4)[:, :, 0:64]
                nc.vector.tensor_copy(
                    dst, pv[:, 0:256].rearrange("p (h c) -> p h c", h=4)
                )

            def emit_scores_tk(h, tk):
                # h is the core-local head index 0..3; buffer = h % 2
                hp, half = h // 2, h % 2
                r0 = 64 * half
                krow = kT2[hp][r0 : r0 + 64, :]
                qrow = qT2[hp][r0 : r0 + 64, :]
                span = SPAN[tk]
                kt = krow[:, 128 * tk : 128 * tk + 128]
                for part in range(math.ceil(span / 1024)):
                    pspan = min(1024, span - 1024 * part)
                    pt = sps.tile([128, 1024], F32, name="sps_t", tag="s")
                    for mmi in range(math.ceil(pspan / 512)):
                        n = min(512, pspan - 512 * mmi)
                        tq0 = 128 * tk + 1024 * part + 512 * mmi
                        nc.tensor.matmul(
                            pt[:, 512 * mmi : 512 * mmi + n],
                            kt,
                            qrow[:, tq0 : tq0 + n],
                            start=True,
                            stop=(part > 0 or mmi > 0),
                            skip_group_check=True,
                        )
                        if part == 0 and mmi == 0:
                            # causal mask: accumulate -BIGL above the diagonal
                            nc.tensor.matmul(
                                pt[:, 0:128],
                                ub_sb[:],
                                id_sb[:],
                                start=False,
                                stop=True,
                                skip_group_check=True,
                            )
                    dst = ab[ABMAP[h]][
                        :, OFF[tk] + 1024 * part : OFF[tk] + 1024 * part + pspan
                    ]
                    nc.scalar.activation(
                        dst,
                        pt[:, 0:pspan],
                        mybir.ActivationFunctionType.Exp,
                        scale=ESCALE,
                    )

            def emit_y_t(h, t, st):
                """Flipped y for tq-tile t: yp[tq, s|d] = sum_tk p.T @ [v|1];
                normalize into staging tile st cols [64*(h%2) : +64]."""
                buf = ab[ABMAP[h]][:]
                yp = yps.tile([128, 65], F32, name="yps_t", tag="y")
                for tk in range(t + 1):
                    c = OFF[tk] + 128 * (t - tk)
                    nc.tensor.matmul(
                        yp[:],
                        buf[:, c : c + 128],
                        v_sb[tk].rearrange("p (h c) -> p h c", h=4)[:, h],
                        start=(tk == 0),
                        stop=(tk == t),
                        skip_group_check=True,
                    )
                rec = smp.tile([128, 1], F32, name="rec")
                nc.vector.reciprocal(rec[:], yp[:, 64:65])
                nc.vector.tensor_scalar_mul(
                    st[:, 64 * (h % 2) : 64 * (h % 2) + 64], yp[:, 0:64], rec[:]
                )

            def emit_y_pair(hp, t):
                """y for both heads of the pair into a staging tile."""
                st = smp.tile([128, 128], BF16, name="st")
                for h in (2 * hp, 2 * hp + 1):
                    emit_y_t(h, t, st)
                return st

            def emit_yt_transpose(hp, t, st):
                tp = mps.tile([128, 128], BF16, name="mp_t", tag="mp")
                nc.tensor.transpose(tp[:], st[:], id_sb[:])
                nc.vector.tensor_copy(yT_all[hp][:, 128 * t : 128 * t + 128], tp[:])

            def emit_proj_t(t, late):
                pps_t = {}
                for n in range(2):
                    # tail tiles draw psum from the scores pool, which is idle
                    # once the exp stream ends: deeper ring, no serialization
                    # against the transpose tiles
                    if late:
                        pp = sps.tile([128, 1024], F32, name="warm", tag="s")[:, 0:512]
                    else:
                        pp = mps.tile([128, 512], F32, name="mp_t", tag="mp")
                    pps_t[n] = pp
                    nc.tensor.matmul(
                        pp[:],
                        yT_all[0][:, 128 * t : 128 * t + 128],
                        wpT_sb[0][:, 512 * n : 512 * n + 512],
                        start=True,
                        stop=False,
                        skip_group_check=True,
                    )
                ot = outs.tile([128, 1024], BF16, name="ot")
                for n in range(2):
                    pp = pps_t[n]
                    nc.tensor.matmul(
                        pp[:],
                        yT_all[1][:, 128 * t : 128 * t + 128],
                        wpT_sb[1][:, 512 * n : 512 * n + 512],
                        start=False,
                        stop=True,
                        skip_group_check=True,
                    )
                    # tail tiles: ACT is free once the exp stream ends
                    if late and n == 1:
                        nc.scalar.copy(ot[:, 512 * n : 512 * n + 512], pp[:])
                    else:
                        nc.vector.tensor_copy(
                            ot[:, 512 * n : 512 * n + 512], pp[:]
                        )
                eng = nc.gpsimd if t >= 14 else nc.sync
                eng.dma_start(out_d[128 * t : 128 * t + 128, :], ot[:])

            # PE warm-up on the ub tile while input DMAs land
            warm = sps.tile([128, 1024], F32, name="warm", tag="s")
            for i in range(24):
                nc.tensor.matmul(
                    warm[:, 0:128], ub_sb[:], ub_sb[:], start=True, stop=True
                )

            # ---- phase A: heads 0-1 ----
            for tq in range(4):
                emit_qk_group(0, 0, tq)
                emit_qk_group(0, 1, tq)
            # head-pair-1 QKV groups are drip-fed between score tiles of
            # rounds 2-3 so no single PE detour outruns the exp backlog
            qk1_queue = [(1, kind, tq) for tq in range(4) for kind in (0, 1)]
            for g in range(4):
                for h in (0, 1):
                    for tk in range(4 * g, 4 * g + 4):
                        emit_scores_tk(h, tk)
                        if h == 0:
                            emit_v_t(tk)
                        elif g >= 2 and qk1_queue:
                            emit_qk_group(*qk1_queue.pop(0))
                if g > 0:
                    sts = [emit_y_pair(0, t) for t in range(4 * (g - 1), 4 * g)]
                    for i, t in enumerate(range(4 * (g - 1), 4 * g)):
                        emit_yt_transpose(0, t, sts[i])
            sts = [emit_y_pair(0, t) for t in range(12, 16)]
            for i, t in enumerate(range(12, 16)):
                emit_yt_transpose(0, t, sts[i])

            # ---- phase B: heads 2-3 + projection ----
            for g in range(4):
                for h in (2, 3):
                    for tk in range(4 * g, 4 * g + 4):
                        emit_scores_tk(h, tk)
                if g > 0:
                    ts = list(range(4 * (g - 1), 4 * g))
                    sts = [emit_y_pair(1, t) for t in ts]
                    for i, t in enumerate(ts):
                        emit_yt_transpose(1, t, sts[i])
                    for t in ts:
                        emit_proj_t(t, late=False)
            ts = list(range(12, 16))
            sts = [emit_y_pair(1, t) for t in ts]
            for i, t in enumerate(ts):
                emit_yt_transpose(1, t, sts[i])
            for t in ts:
                emit_proj_t(t, late=True)

    nc.finalize()
    return nc


def _get_nc():
    global _cached_nc
    if _cached_nc is None:
        _cached_nc = _build()
    return _cached_nc


def kernel(x, Wq, Wk, Wv, Wp, bp):
    global last_results
    x = np.asarray(x, dtype=np.float32)
    Wq = np.asarray(Wq, dtype=np.float32)
    Wk = np.asarray(Wk, dtype=np.float32)
    Wv = np.asarray(Wv, dtype=np.float32)
    Wp = np.asarray(Wp, dtype=np.float32)
    bp = np.asarray(bp, dtype=np.float32)

    WpT = np.ascontiguousarray(Wp.T) / WSCALE  # [C_in(features), C_out]
    ub = np.triu(np.full((128, 128), -BIGL, dtype=np.float32), k=1).astype(
        ml_dtypes.bfloat16
    )
    id128 = np.eye(128, dtype=np.float32).astype(ml_dtypes.bfloat16)
    f8 = ml_dtypes.float8_e4m3

    def chunked(w):
        # [C, m] -> [128, 8*m]: c-chunk s at cols [m*s : m*(s+1)]
        m = w.shape[1]
        return np.ascontiguousarray(
            w.reshape(8, 128, m).transpose(1, 0, 2).reshape(128, 8 * m)
        )

    def hi_lo(w):
        hi = w.astype(f8)
        lo = (w - hi.astype(np.float32)).astype(f8)
        return hi, lo

    x8_by_b, xl_by_b = [], []
    for b in range(B):
        hi, lo = hi_lo(chunked(np.ascontiguousarray(x[b].T)))
        x8_by_b.append(hi)
        xl_by_b.append(lo)

    in_maps = []
    for core in range(N_CORES):
        b, g = core // 4, core % 4
        h0 = HPC * g
        wq_c = np.stack([chunked(np.concatenate(
            [Wq[h0 + 2 * hp] * WSCALE, Wq[h0 + 2 * hp + 1] * WSCALE], axis=1))
            for hp in range(2)])
        wk_c = np.stack([chunked(np.concatenate(
            [Wk[h0 + 2 * hp] * WSCALE, Wk[h0 + 2 * hp + 1] * WSCALE], axis=1))
            for hp in range(2)])
        wv_c = chunked(np.concatenate(
            [Wv[h0 + j] * WSCALE for j in range(HPC)], axis=1))
        wq_hi, wq_lo = hi_lo(wq_c)
        wk_hi, wk_lo = hi_lo(wk_c)
        wv_hi, wv_lo = hi_lo(wv_c)
        wpT_p = np.ascontiguousarray(
            WpT[256 * g : 256 * (g + 1)].reshape(2, 128, C)
        ).astype(ml_dtypes.bfloat16)
        in_maps.append({
            "xT": x8_by_b[b], "xl": xl_by_b[b],
            "wq": wq_hi, "wql": wq_lo,
            "wk": wk_hi, "wkl": wk_lo,
            "wv": wv_hi, "wvl": wv_lo,
            "wpT": wpT_p, "ub": ub, "id": id128,
        })

    nc = _get_nc()
    kwargs = {}
    if os.environ.get("KERNEL_TRACE", "0") == "1":
        kwargs = dict(trace=True, trace_cores=list(range(N_CORES)),
                      stitch_traces=True)
    try:
        res = run_bass_kernel_spmd(
            nc, in_maps, core_ids=list(range(N_CORES)), **kwargs
        )
    except ModuleNotFoundError:
        res = run_bass_kernel_spmd(nc, in_maps, core_ids=list(range(N_CORES)))
    last_results = res

    out = np.zeros((B, T, C), dtype=np.float32)
    for core in range(N_CORES):
        b = core // 4
        out[b] += res.results[core]["out"].astype(np.float32)
    out += bp[None, None, :]
    return out


# revision 32
# speedup vs baseline: 1.0683x; 1.0512x over previous
"""Multi-head causal attention (B=2, T=2048, C=1024, H=16, S=64) on 8 TRN2 cores.

Sharding: core i handles batch b = i//4 and head group g = i%4 (4 heads each).
Each core computes a partial output projection (its heads' contribution to the
full [T, C] output); the host sums the 4 partials per batch and adds the bias.

Per-core dataflow (f32 PSUM accumulation throughout):
  qT/kT  [128, T] bf16 = (32W).T @ x   -- fp8 DoubleRow over c-chunk pairs,
         3 passes (hi*hi + hi*Wlo + xlo*hi) so fp8 quantization cancels
  v      [128, 4h x 65] bf16 (32-scaled, ones column for the denominator),
         same 3-pass fp8 DR projection
  attT   [tk, tq] psum = kT-tile.T @ qT (bf16, K=64) with a -1e6 upper-
         triangle matmul accumulated onto the diagonal block (causal mask)
  p      = exp(att/(sqrt(S)*1024))     (ACT, bf16 out; mask underflows to 0)
  y      [tq=128, 65] psum per t-tile = p-tile.T @ [v|1]  (flipped: N=65)
  ynorm  = y * recip(d) per-partition (DVE tensor_scalar), staged per
         head-pair then PE-transposed back to yT [s, t] layout
  out    [T, C] partial = yT.T @ (WpT/32) (bf16, head-pair accumulation)

Heads processed in two phases (pair 0-1 then 2-3) so only two bf16 attT
buffers are live; the projection runs in phase 2, gated per t-tile.
"""

import os
import math
import numpy as np
import ml_dtypes

import concourse.bacc as bacc
import concourse.mybir as mybir
import concourse.tile as tile
from concourse.bass_utils import run_bass_kernel_spmd

F32 = mybir.dt.float32
BF16 = mybir.dt.bfloat16
FP8 = mybir.dt.float8e4
DR = mybir.MatmulPerfMode.DoubleRow

B, T, C, H, S = 2, 2048, 1024, 16, 64
HPC = 4          # heads per core
N_CORES = 8
NC_T = T // 128  # 16 t-tiles of 128

WSCALE = 32.0                       # weight scale (fp8 denormal avoidance)
ESCALE = 0.125 / (WSCALE * WSCALE)  # exp scale on (32q)(32k) psum logits
BIGL = 1.0e6                        # causal-mask logit offset (pre-ESCALE)

# attT storage offsets: tile tk spans tq in [128*tk, 2048)
SPAN = [T - 128 * tk for tk in range(NC_T)]
OFF = [0] * NC_T
for _tk in range(1, NC_T):
    OFF[_tk] = OFF[_tk - 1] + SPAN[_tk - 1]
ATT_W = OFF[-1] + SPAN[-1]  # 17408

_cached_nc = None
last_results = None  # BassKernelResults of the most recent run (for test harness)


def _build():
    nc = bacc.Bacc("TRN2", target_bir_lowering=False)

    # contraction row c = 128*s + p lives at partition p, slot s.
    # *l tensors are the fp8 quantization residuals.
    xT_d = nc.dram_tensor("xT", [128, 8 * T], FP8, kind="ExternalInput")
    xl_d = nc.dram_tensor("xl", [128, 8 * T], FP8, kind="ExternalInput")
    wq_d = nc.dram_tensor("wq", [2, 128, 8 * 128], FP8, kind="ExternalInput")
    wk_d = nc.dram_tensor("wk", [2, 128, 8 * 128], FP8, kind="ExternalInput")
    wql_d = nc.dram_tensor("wql", [2, 128, 8 * 128], FP8, kind="ExternalInput")
    wkl_d = nc.dram_tensor("wkl", [2, 128, 8 * 128], FP8, kind="ExternalInput")
    wv_d = nc.dram_tensor("wv", [128, 8 * 256], FP8, kind="ExternalInput")
    wvl_d = nc.dram_tensor("wvl", [128, 8 * 256], FP8, kind="ExternalInput")
    wpT_d = nc.dram_tensor("wpT", [2, 128, C], BF16, kind="ExternalInput")
    ub_d = nc.dram_tensor("ub", [128, 128], BF16, kind="ExternalInput")
    id_d = nc.dram_tensor("id", [128, 128], BF16, kind="ExternalInput")
    out_d = nc.dram_tensor("out", [T, C], BF16, kind="ExternalOutput")

    with tile.TileContext(nc) as tc:
        with (
            tc.tile_pool(name="const", bufs=1) as constp,
            tc.tile_pool(name="qkT", bufs=1) as qkp,
            tc.tile_pool(name="vsb", bufs=1) as vp,
            tc.tile_pool(name="yT", bufs=1) as ytp,
            tc.tile_pool(name="attT", bufs=1) as attp,
            tc.tile_pool(name="sps", bufs=3, space="PSUM") as sps,
            tc.tile_pool(name="mps", bufs=2, space="PSUM") as mps,
            tc.tile_pool(name="xw", bufs=1) as xw,
            tc.tile_pool(name="outs", bufs=4) as outs,
            tc.tile_pool(name="sm", bufs=5) as smp,
        ):
            ub_sb = constp.tile([128, 128], BF16, name="ub_sb")
            id_sb = constp.tile([128, 128], BF16, name="id_sb")
            nc.gpsimd.dma_start(ub_sb[:], ub_d[:])
            nc.gpsimd.dma_start(id_sb[:], id_d[:])
            wpT_sb = [
                constp.tile([128, C], BF16, name=f"wpT{hp}") for hp in range(2)
            ]

            qT2 = [qkp.tile([128, T], BF16, name=f"qT2_{hp}") for hp in range(2)]
            kT2 = [qkp.tile([128, T], BF16, name=f"kT2_{hp}") for hp in range(2)]
            # v tiles: [128, 4 heads x 65] bf16; col 64 of each 65-block = 1
            v_sb = [vp.tile([128, 4 * 65], BF16, name=f"v{t}") for t in range(NC_T)]
            for t in range(NC_T):
                nc.vector.memset(
                    v_sb[t].rearrange("p (h c) -> p h c", h=4)[:, :, 64], 1.0
                )
            yT_all = [ytp.tile([128, T], BF16, name=f"yTa{hp}") for hp in range(2)]
            # three attT buffers: h0->0, h1->1, h2->2, h3->0. The third
            # lets phase B's first head start before phase A's y tail drains.
            ab = [attp.tile([128, ATT_W], BF16, name=f"attb{i}") for i in range(3)]
            ABMAP = {0: 0, 1: 1, 2: 2, 3: 0}

            # input DMAs: x8 pairs + hp0 hi weights first, then residuals
            wq_sb = [xw.tile([128, 1024], FP8, name=f"wq{hp}") for hp in range(2)]
            wk_sb = [xw.tile([128, 1024], FP8, name=f"wk{hp}") for hp in range(2)]
            wql_sb = [xw.tile([128, 1024], FP8, name=f"wql{hp}") for hp in range(2)]
            wkl_sb = [xw.tile([128, 1024], FP8, name=f"wkl{hp}") for hp in range(2)]
            wv_sb = xw.tile([128, 2048], FP8, name="wv")
            wvl_sb = xw.tile([128, 2048], FP8, name="wvl")
            xT_sb = xw.tile([128, 8 * T], FP8, name="xT")
            xl_sb = xw.tile([128, 8 * T], FP8, name="xl")

            nc.sync.dma_start(xT_sb[:, 0:4096], xT_d[:, 0:4096])
            nc.sync.dma_start(wq_sb[0][:], wq_d[0])
            nc.sync.dma_start(wk_sb[0][:], wk_d[0])
            for sp in range(1, 4):
                nc.sync.dma_start(
                    xT_sb[:, 4096 * sp : 4096 * sp + 4096],
                    xT_d[:, 4096 * sp : 4096 * sp + 4096],
                )
            nc.sync.dma_start(wql_sb[0][:], wql_d[0])
            nc.sync.dma_start(wkl_sb[0][:], wkl_d[0])
            for sp in range(4):
                nc.sync.dma_start(
                    xl_sb[:, 4096 * sp : 4096 * sp + 4096],
                    xl_d[:, 4096 * sp : 4096 * sp + 4096],
                )
            nc.sync.dma_start(wv_sb[:], wv_d[:])
            nc.sync.dma_start(wvl_sb[:], wvl_d[:])
            nc.sync.dma_start(wq_sb[1][:], wq_d[1])
            nc.sync.dma_start(wk_sb[1][:], wk_d[1])
            nc.sync.dma_start(wql_sb[1][:], wql_d[1])
            nc.sync.dma_start(wkl_sb[1][:], wkl_d[1])
            for hp in range(2):
                nc.gpsimd.dma_start(wpT_sb[hp][:], wpT_d[hp])

            x_sl = xT_sb.rearrange("p (s t) -> p s t", s=8)
            xl_sl = xl_sb.rearrange("p (s t) -> p s t", s=8)

            def emit_qk_group(hp, kind, tq):
                w_hi = (wq_sb if kind == 0 else wk_sb)[hp]
                w_lo = (wql_sb if kind == 0 else wkl_sb)[hp]
                dst = (qT2 if kind == 0 else kT2)[hp]
                pt = mps.tile([128, 512], F32, name="mp_t", tag="mp")
                passes = ((w_hi, x_sl), (w_lo, x_sl), (w_hi, xl_sl))
                for pi, (wgt, xs) in enumerate(passes):
                    for jj in range(4):
                        lhsT = wgt[:, 256 * jj : 256 * jj + 256].rearrange(
                            "p (j m) -> p j m", j=2
                        )
                        rhs = xs[:, 2 * jj : 2 * jj + 2, 512 * tq : 512 * tq + 512]
                        nc.tensor.matmul(
                            pt[:],
                            lhsT,
                            rhs,
                            start=(pi == 0 and jj == 0),
                            stop=(pi == 2 and jj == 3),
                            perf_mode=DR,
                            skip_group_check=True,
                        )
                # hp0 q copies on ACT (idle pre-stream); rest on DVE
                if kind == 0 and hp == 0:
                    nc.scalar.copy(dst[:, 512 * tq : 512 * tq + 512], pt[:])
                else:
                    nc.vector.tensor_copy(dst[:, 512 * tq : 512 * tq + 512], pt[:])

            def emit_v_t(t):
                pv = mps.tile([128, 512], F32, name="mp_t", tag="mp")
                passes = ((wv_sb, x_sl), (wvl_sb, x_sl), (wv_sb, xl_sl))
                for pi, (wgt, xs) in enumerate(passes):
                    for jj in range(4):
                        lhsT = xs[:, 2 * jj : 2 * jj + 2, 128 * t : 128 * t + 128]
                        rhs = wgt[:, 512 * jj : 512 * jj + 512].rearrange(
                            "p (j m) -> p j m", j=2
                        )
                        nc.tensor.matmul(
                            pv[:, 0:256],
                            lhsT,
                            rhs,
                            start=(pi == 0 and jj == 0),
                            stop=(pi == 2 and jj == 3),
                            perf_mode=DR,
                            skip_group_check=True,
                        )
                dst = v_sb[t].rearrange("p (h c) -> p h c", h=4)[:, :, 0:64]
                nc.vector.tensor_copy(
                    dst, pv[:, 0:256].rearrange("p (h c) -> p h c", h=4)
                )

            def emit_scores_tk(h, tk):
                # h is the core-local head index 0..3; buffer = h % 2
                hp, half = h // 2, h % 2
                r0 = 64 * half
                krow = kT2[hp][r0 : r0 + 64, :]
                qrow = qT2[hp][r0 : r0 + 64, :]
                span = SPAN[tk]
                kt = krow[:, 128 * tk : 128 * tk + 128]
                for part in range(math.ceil(span / 1024)):
                    pspan = min(1024, span - 1024 * part)
                    pt = sps.tile([128, 1024], F32, name="sps_t", tag="s")
                    for mmi in range(math.ceil(pspan / 512)):
                        n = min(512, pspan - 512 * mmi)
                        tq0 = 128 * tk + 1024 * part + 512 * mmi
                        nc.tensor.matmul(
                            pt[:, 512 * mmi : 512 * mmi + n],
                            kt,
                            qrow[:, tq0 : tq0 + n],
                            start=True,
                            stop=(part > 0 or mmi > 0),
                            skip_group_check=True,
                        )
                        if part == 0 and mmi == 0:
                            # causal mask: accumulate -BIGL above the diagonal
                            nc.tensor.matmul(
                                pt[:, 0:128],
                                ub_sb[:],
                                id_sb[:],
                                start=False,
                                stop=True,
                                skip_group_check=True,
                            )
                    dst = ab[ABMAP[h]][
                        :, OFF[tk] + 1024 * part : OFF[tk] + 1024 * part + pspan
                    ]
                    nc.scalar.activation(
                        dst,
                        pt[:, 0:pspan],
                        mybir.ActivationFunctionType.Exp,
                        scale=ESCALE,
                    )

            def emit_y_t(h, t, st, late=False):
                """Flipped y for tq-tile t: yp[tq, s|d] = sum_tk p.T @ [v|1];
                normalize into staging tile st cols [64*(h%2) : +64]."""
                buf = ab[ABMAP[h]][:]
                if late:
                    yp = sps.tile([128, 512], F32, name="warm", tag="s")[:, 0:65]
                else:
                    yp = mps.tile([128, 512], F32, name="mp_t", tag="mp")[:, 0:65]
                for tk in range(t + 1):
                    c = OFF[tk] + 128 * (t - tk)
                    nc.tensor.matmul(
                        yp[:],
                        buf[:, c : c + 128],
                        v_sb[tk].rearrange("p (h c) -> p h c", h=4)[:, h],
                        start=(tk == 0),
                        stop=(tk == t),
                        skip_group_check=True,
                    )
                rec = smp.tile([128, 1], F32, name="rec")
                nc.vector.reciprocal(rec[:], yp[:, 64:65])
                nc.vector.tensor_scalar_mul(
                    st[:, 64 * (h % 2) : 64 * (h % 2) + 64], yp[:, 0:64], rec[:]
                )

            def emit_y_pair(hp, t, late=False):
                """y for both heads of the pair into a staging tile."""
                st = smp.tile([128, 128], BF16, name="st")
                for h in (2 * hp, 2 * hp + 1):
                    emit_y_t(h, t, st, late)
                return st

            def emit_yt_transpose(hp, t, st, late=False):
                tp = mps.tile([128, 128], BF16, name="mp_t", tag="mp")
                nc.tensor.transpose(tp[:], st[:], id_sb[:])
                nc.vector.tensor_copy(yT_all[hp][:, 128 * t : 128 * t + 128], tp[:])

            def emit_proj_t(t, late):
                pps_t = {}
                for n in range(2):
                    # tail tiles draw psum from the scores pool, which is idle
                    # once the exp stream ends: deeper effective ring
                    if late:
                        pp = sps.tile([128, 1024], F32, name="warm", tag="s")[:, 0:512]
                    else:
                        pp = mps.tile([128, 512], F32, name="mp_t", tag="mp")
                    pps_t[n] = pp
                    nc.tensor.matmul(
                        pp[:],
                        yT_all[0][:, 128 * t : 128 * t + 128],
                        wpT_sb[0][:, 512 * n : 512 * n + 512],
                        start=True,
                        stop=False,
                        skip_group_check=True,
                    )
                ot = outs.tile([128, 1024], BF16, name="ot")
                for n in range(2):
                    pp = pps_t[n]
                    nc.tensor.matmul(
                        pp[:],
                        yT_all[1][:, 128 * t : 128 * t + 128],
                        wpT_sb[1][:, 512 * n : 512 * n + 512],
                        start=False,
                        stop=True,
                        skip_group_check=True,
                    )
                    # tail tiles: ACT is free once the exp stream ends
                    if late and n == 1:
                        nc.scalar.copy(ot[:, 512 * n : 512 * n + 512], pp[:])
                    else:
                        nc.vector.tensor_copy(
                            ot[:, 512 * n : 512 * n + 512], pp[:]
                        )
                eng = nc.gpsimd if t >= 14 else nc.sync
                eng.dma_start(out_d[128 * t : 128 * t + 128, :], ot[:])

            # PE warm-up on the ub tile while input DMAs land
            warm = sps.tile([128, 1024], F32, name="warm", tag="s")
            for i in range(24):
                nc.tensor.matmul(
                    warm[:, 0:128], ub_sb[:], ub_sb[:], start=True, stop=True
                )

            # ---- phase A: heads 0-1 ----
            for tq in range(4):
                emit_qk_group(0, 0, tq)
                emit_qk_group(0, 1, tq)
            # head-pair-1 QKV groups are drip-fed between score tiles of
            # rounds 2-3 so no single PE detour outruns the exp backlog
            qk1_queue = [(1, kind, tq) for tq in range(4) for kind in (0, 1)]
            for g in range(4):
                for h in (0, 1):
                    for tk in range(4 * g, 4 * g + 4):
                        emit_scores_tk(h, tk)
                        if h == 0:
                            emit_v_t(tk)
                        elif g >= 2 and qk1_queue:
                            emit_qk_group(*qk1_queue.pop(0))
                if g > 0:
                    sts = [emit_y_pair(0, t) for t in range(4 * (g - 1), 4 * g)]
                    for i, t in enumerate(range(4 * (g - 1), 4 * g)):
                        emit_yt_transpose(0, t, sts[i])
            sts = [emit_y_pair(0, t) for t in range(12, 16)]
            for i, t in enumerate(range(12, 16)):
                emit_yt_transpose(0, t, sts[i])

            # ---- phase B: heads 2-3 + projection ----
            for g in range(4):
                for h in (2, 3):
                    for tk in range(4 * g, 4 * g + 4):
                        emit_scores_tk(h, tk)
                if g > 0:
                    ts = list(range(4 * (g - 1), 4 * g))
                    sts = [emit_y_pair(1, t) for t in ts]
                    for i, t in enumerate(ts):
                        emit_yt_transpose(1, t, sts[i])
                    for t in ts:
                        emit_proj_t(t, late=False)
            ts = list(range(12, 16))
            sts = [emit_y_pair(1, t, late=True) for t in ts]
            for i, t in enumerate(ts):
                emit_yt_transpose(1, t, sts[i], late=True)
            for t in ts:
                emit_proj_t(t, late=True)

    nc.finalize()
    return nc


def _get_nc():
    global _cached_nc
    if _cached_nc is None:
        _cached_nc = _build()
    return _cached_nc


def kernel(x, Wq, Wk, Wv, Wp, bp):
    global last_results
    x = np.asarray(x, dtype=np.float32)
    Wq = np.asarray(Wq, dtype=np.float32)
    Wk = np.asarray(Wk, dtype=np.float32)
    Wv = np.asarray(Wv, dtype=np.float32)
    Wp = np.asarray(Wp, dtype=np.float32)
    bp = np.asarray(bp, dtype=np.float32)

    WpT = np.ascontiguousarray(Wp.T) / WSCALE  # [C_in(features), C_out]
    ub = np.triu(np.full((128, 128), -BIGL, dtype=np.float32), k=1).astype(
        ml_dtypes.bfloat16
    )
    id128 = np.eye(128, dtype=np.float32).astype(ml_dtypes.bfloat16)
    f8 = ml_dtypes.float8_e4m3

    def chunked(w):
        # [C, m] -> [128, 8*m]: c-chunk s at cols [m*s : m*(s+1)]
        m = w.shape[1]
        return np.ascontiguousarray(
            w.reshape(8, 128, m).transpose(1, 0, 2).reshape(128, 8 * m)
        )

    def hi_lo(w):
        hi = w.astype(f8)
        lo = (w - hi.astype(np.float32)).astype(f8)
        return hi, lo

    x8_by_b, xl_by_b = [], []
    for b in range(B):
        hi, lo = hi_lo(chunked(np.ascontiguousarray(x[b].T)))
        x8_by_b.append(hi)
        xl_by_b.append(lo)

    in_maps = []
    for core in range(N_CORES):
        b, g = core // 4, core % 4
        h0 = HPC * g
        wq_c = np.stack([chunked(np.concatenate(
            [Wq[h0 + 2 * hp] * WSCALE, Wq[h0 + 2 * hp + 1] * WSCALE], axis=1))
            for hp in range(2)])
        wk_c = np.stack([chunked(np.concatenate(
            [Wk[h0 + 2 * hp] * WSCALE, Wk[h0 + 2 * hp + 1] * WSCALE], axis=1))
            for hp in range(2)])
        wv_c = chunked(np.concatenate(
            [Wv[h0 + j] * WSCALE for j in range(HPC)], axis=1))
        wq_hi, wq_lo = hi_lo(wq_c)
        wk_hi, wk_lo = hi_lo(wk_c)
        wv_hi, wv_lo = hi_lo(wv_c)
        wpT_p = np.ascontiguousarray(
            WpT[256 * g : 256 * (g + 1)].reshape(2, 128, C)
        ).astype(ml_dtypes.bfloat16)
        in_maps.append({
            "xT": x8_by_b[b], "xl": xl_by_b[b],
            "wq": wq_hi, "wql": wq_lo,
            "wk": wk_hi, "wkl": wk_lo,
            "wv": wv_hi, "wvl": wv_lo,
            "wpT": wpT_p, "ub": ub, "id": id128,
        })

    nc = _get_nc()
    kwargs = {}
    if os.environ.get("KERNEL_TRACE", "0") == "1":
        kwargs = dict(trace=True, trace_cores=list(range(N_CORES)),
                      stitch_traces=True)
    try:
        res = run_bass_kernel_spmd(
            nc, in_maps, core_ids=list(range(N_CORES)), **kwargs
        )
    except ModuleNotFoundError:
        res = run_bass_kernel_spmd(nc, in_maps, core_ids=list(range(N_CORES)))
    last_results = res

    out = np.zeros((B, T, C), dtype=np.float32)
    for core in range(N_CORES):
        b = core // 4
        out[b] += res.results[core]["out"].astype(np.float32)
    out += bp[None, None, :]
    return out


# revision 35
# speedup vs baseline: 1.0875x; 1.0179x over previous
"""Multi-head causal attention (B=2, T=2048, C=1024, H=16, S=64) on 8 TRN2 cores.

Sharding: core i handles batch b = i//4 and head group g = i%4 (4 heads each).
Each core computes a partial output projection (its heads' contribution to the
full [T, C] output); the host sums the 4 partials per batch and adds the bias.

Per-core dataflow (f32 PSUM accumulation throughout):
  qT/kT  [128, T] bf16 = (32W).T @ x   -- fp8 DoubleRow over c-chunk pairs,
         3 passes (hi*hi + hi*Wlo + xlo*hi) so fp8 quantization cancels
  v      [128, 4h x 65] bf16 (32-scaled, ones column for the denominator),
         same 3-pass fp8 DR projection
  attT   [tk, tq] psum = kT-tile.T @ qT (bf16, K=64) with a -1e6 upper-
         triangle matmul accumulated onto the diagonal block (causal mask)
  p      = exp(att/(sqrt(S)*1024))     (ACT, bf16 out; mask underflows to 0)
  y      [tq=128, 65] psum per t-tile = p-tile.T @ [v|1]  (flipped: N=65)
  ynorm  = y * recip(d) per-partition (DVE tensor_scalar), staged per
         head-pair then PE-transposed back to yT [s, t] layout
  out    [T, C] partial = yT.T @ (WpT/32) (bf16, head-pair accumulation)

Heads processed in two phases (pair 0-1 then 2-3) so only two bf16 attT
buffers are live; the projection runs in phase 2, gated per t-tile.
"""

import os
import math
import numpy as np
import ml_dtypes

import concourse.bacc as bacc
import concourse.mybir as mybir
import concourse.tile as tile
from concourse.bass_utils import run_bass_kernel_spmd

F32 = mybir.dt.float32
BF16 = mybir.dt.bfloat16
FP8 = mybir.dt.float8e4
DR = mybir.MatmulPerfMode.DoubleRow

B, T, C, H, S = 2, 2048, 1024, 16, 64
HPC = 4          # heads per core
N_CORES = 8
NC_T = T // 128  # 16 t-tiles of 128

WSCALE = 32.0                       # weight scale (fp8 denormal avoidance)
ESCALE = 0.125 / (WSCALE * WSCALE)  # exp scale on (32q)(32k) psum logits
BIGL = 1.0e6                        # causal-mask logit offset (pre-ESCALE)

# attT storage offsets: tile tk spans tq in [128*tk, 2048)
SPAN = [T - 128 * tk for tk in range(NC_T)]
OFF = [0] * NC_T
for _tk in range(1, NC_T):
    OFF[_tk] = OFF[_tk - 1] + SPAN[_tk - 1]
ATT_W = OFF[-1] + SPAN[-1]  # 17408

_cached_nc = None
last_results = None  # BassKernelResults of the most recent run (for test harness)


def _build():
    nc = bacc.Bacc("TRN2", target_bir_lowering=False)

    # contraction row c = 128*s + p lives at partition p, slot s.
    # *l tensors are the fp8 quantization residuals.
    xT_d = nc.dram_tensor("xT", [128, 8 * T], FP8, kind="ExternalInput")
    xl_d = nc.dram_tensor("xl", [128, 8 * T], FP8, kind="ExternalInput")
    wq_d = nc.dram_tensor("wq", [2, 128, 8 * 128], FP8, kind="ExternalInput")
    wk_d = nc.dram_tensor("wk", [2, 128, 8 * 128], FP8, kind="ExternalInput")
    wql_d = nc.dram_tensor("wql", [2, 128, 8 * 128], FP8, kind="ExternalInput")
    wkl_d = nc.dram_tensor("wkl", [2, 128, 8 * 128], FP8, kind="ExternalInput")
    wv_d = nc.dram_tensor("wv", [128, 8 * 256], FP8, kind="ExternalInput")
    wvl_d = nc.dram_tensor("wvl", [128, 8 * 256], FP8, kind="ExternalInput")
    wpT_d = nc.dram_tensor("wpT", [2, 128, C], BF16, kind="ExternalInput")
    ub_d = nc.dram_tensor("ub", [128, 128], BF16, kind="ExternalInput")
    id_d = nc.dram_tensor("id", [128, 128], BF16, kind="ExternalInput")
    out_d = nc.dram_tensor("out", [T, C], BF16, kind="ExternalOutput")

    with tile.TileContext(nc) as tc:
        with (
            tc.tile_pool(name="const", bufs=1) as constp,
            tc.tile_pool(name="qkT", bufs=1) as qkp,
            tc.tile_pool(name="vsb", bufs=1) as vp,
            tc.tile_pool(name="yT", bufs=1) as ytp,
            tc.tile_pool(name="attT", bufs=1) as attp,
            tc.tile_pool(name="sps", bufs=3, space="PSUM") as sps,
            tc.tile_pool(name="mps", bufs=2, space="PSUM") as mps,
            tc.tile_pool(name="xw", bufs=1) as xw,
            tc.tile_pool(name="outs", bufs=4) as outs,
            tc.tile_pool(name="sm", bufs=5) as smp,
        ):
            ub_sb = constp.tile([128, 128], BF16, name="ub_sb")
            id_sb = constp.tile([128, 128], BF16, name="id_sb")
            nc.gpsimd.dma_start(ub_sb[:], ub_d[:])
            nc.gpsimd.dma_start(id_sb[:], id_d[:])
            wpT_sb = [
                constp.tile([128, C], BF16, name=f"wpT{hp}") for hp in range(2)
            ]

            qT2 = [qkp.tile([128, T], BF16, name=f"qT2_{hp}") for hp in range(2)]
            kT2 = [qkp.tile([128, T], BF16, name=f"kT2_{hp}") for hp in range(2)]
            # v tiles: [128, 4 heads x 65] bf16; col 64 of each 65-block = 1
            v_sb = [vp.tile([128, 4 * 65], BF16, name=f"v{t}") for t in range(NC_T)]
            for t in range(NC_T):
                nc.vector.memset(
                    v_sb[t].rearrange("p (h c) -> p h c", h=4)[:, :, 64], 1.0
                )
            yT_all = [ytp.tile([128, T], BF16, name=f"yTa{hp}") for hp in range(2)]
            # three attT buffers: h0->0, h1->1, h2->2, h3->0. The third
            # lets phase B's first head start before phase A's y tail drains.
            ab = [attp.tile([128, ATT_W], BF16, name=f"attb{i}") for i in range(3)]
            ABMAP = {0: 0, 1: 1, 2: 2, 3: 0}

            # input DMAs: x8 pairs + hp0 hi weights first, then residuals
            wq_sb = [xw.tile([128, 1024], FP8, name=f"wq{hp}") for hp in range(2)]
            wk_sb = [xw.tile([128, 1024], FP8, name=f"wk{hp}") for hp in range(2)]
            wql_sb = [xw.tile([128, 1024], FP8, name=f"wql{hp}") for hp in range(2)]
            wkl_sb = [xw.tile([128, 1024], FP8, name=f"wkl{hp}") for hp in range(2)]
            wv_sb = xw.tile([128, 2048], FP8, name="wv")
            wvl_sb = xw.tile([128, 2048], FP8, name="wvl")
            xT_sb = xw.tile([128, 8 * T], FP8, name="xT")
            xl_sb = xw.tile([128, 8 * T], FP8, name="xl")

            nc.sync.dma_start(xT_sb[:, 0:4096], xT_d[:, 0:4096])
            nc.sync.dma_start(wq_sb[0][:], wq_d[0])
            nc.sync.dma_start(wk_sb[0][:], wk_d[0])
            for sp in range(1, 4):
                nc.sync.dma_start(
                    xT_sb[:, 4096 * sp : 4096 * sp + 4096],
                    xT_d[:, 4096 * sp : 4096 * sp + 4096],
                )
            nc.sync.dma_start(wql_sb[0][:], wql_d[0])
            nc.sync.dma_start(wkl_sb[0][:], wkl_d[0])
            for sp in range(4):
                nc.sync.dma_start(
                    xl_sb[:, 4096 * sp : 4096 * sp + 4096],
                    xl_d[:, 4096 * sp : 4096 * sp + 4096],
                )
            nc.sync.dma_start(wv_sb[:], wv_d[:])
            nc.sync.dma_start(wvl_sb[:], wvl_d[:])
            nc.sync.dma_start(wq_sb[1][:], wq_d[1])
            nc.sync.dma_start(wk_sb[1][:], wk_d[1])
            nc.sync.dma_start(wql_sb[1][:], wql_d[1])
            nc.sync.dma_start(wkl_sb[1][:], wkl_d[1])
            for hp in range(2):
                nc.gpsimd.dma_start(wpT_sb[hp][:], wpT_d[hp])

            x_sl = xT_sb.rearrange("p (s t) -> p s t", s=8)
            xl_sl = xl_sb.rearrange("p (s t) -> p s t", s=8)

            def emit_qk_group(hp, kind, tq):
                w_hi = (wq_sb if kind == 0 else wk_sb)[hp]
                w_lo = (wql_sb if kind == 0 else wkl_sb)[hp]
                dst = (qT2 if kind == 0 else kT2)[hp]
                pt = mps.tile([128, 512], F32, name="mp_t", tag="mp")
                passes = ((w_hi, x_sl), (w_lo, x_sl), (w_hi, xl_sl))
                for pi, (wgt, xs) in enumerate(passes):
                    for jj in range(4):
                        lhsT = wgt[:, 256 * jj : 256 * jj + 256].rearrange(
                            "p (j m) -> p j m", j=2
                        )
                        rhs = xs[:, 2 * jj : 2 * jj + 2, 512 * tq : 512 * tq + 512]
                        nc.tensor.matmul(
                            pt[:],
                            lhsT,
                            rhs,
                            start=(pi == 0 and jj == 0),
                            stop=(pi == 2 and jj == 3),
                            perf_mode=DR,
                            skip_group_check=True,
                        )
                # hp0 q copies on ACT (idle pre-stream); rest on DVE
                if kind == 0 and hp == 0:
                    nc.scalar.copy(dst[:, 512 * tq : 512 * tq + 512], pt[:])
                else:
                    nc.vector.tensor_copy(dst[:, 512 * tq : 512 * tq + 512], pt[:])

            def emit_v_t(t):
                pv = mps.tile([128, 512], F32, name="mp_t", tag="mp")
                passes = ((wv_sb, x_sl), (wvl_sb, x_sl), (wv_sb, xl_sl))
                for pi, (wgt, xs) in enumerate(passes):
                    for jj in range(4):
                        lhsT = xs[:, 2 * jj : 2 * jj + 2, 128 * t : 128 * t + 128]
                        rhs = wgt[:, 512 * jj : 512 * jj + 512].rearrange(
                            "p (j m) -> p j m", j=2
                        )
                        nc.tensor.matmul(
                            pv[:, 0:256],
                            lhsT,
                            rhs,
                            start=(pi == 0 and jj == 0),
                            stop=(pi == 2 and jj == 3),
                            perf_mode=DR,
                            skip_group_check=True,
                        )
                dst = v_sb[t].rearrange("p (h c) -> p h c", h=4)[:, :, 0:64]
                nc.vector.tensor_copy(
                    dst, pv[:, 0:256].rearrange("p (h c) -> p h c", h=4)
                )

            def emit_scores_tk(h, tk):
                # h is the core-local head index 0..3; buffer = h % 2
                hp, half = h // 2, h % 2
                r0 = 64 * half
                krow = kT2[hp][r0 : r0 + 64, :]
                qrow = qT2[hp][r0 : r0 + 64, :]
                span = SPAN[tk]
                kt = krow[:, 128 * tk : 128 * tk + 128]
                for part in range(math.ceil(span / 1024)):
                    pspan = min(1024, span - 1024 * part)
                    pt = sps.tile([128, 1024], F32, name="sps_t", tag="s")
                    for mmi in range(math.ceil(pspan / 512)):
                        n = min(512, pspan - 512 * mmi)
                        tq0 = 128 * tk + 1024 * part + 512 * mmi
                        nc.tensor.matmul(
                            pt[:, 512 * mmi : 512 * mmi + n],
                            kt,
                            qrow[:, tq0 : tq0 + n],
                            start=True,
                            stop=True,
                            skip_group_check=True,
                        )

                    dst = ab[ABMAP[h]][
                        :, OFF[tk] + 1024 * part : OFF[tk] + 1024 * part + pspan
                    ]
                    nc.scalar.activation(
                        dst,
                        pt[:, 0:pspan],
                        mybir.ActivationFunctionType.Exp,
                        scale=ESCALE,
                    )
                    if part == 0:
                        diag = ab[ABMAP[h]][:, OFF[tk] : OFF[tk] + 128]
                        nc.vector.tensor_mul(diag, diag, ub_sb[:])

            def emit_y_t(h, t, st, late=False):
                """Flipped y for tq-tile t: yp[tq, s|d] = sum_tk p.T @ [v|1];
                normalize into staging tile st cols [64*(h%2) : +64]."""
                buf = ab[ABMAP[h]][:]
                if late:
                    yp = sps.tile([128, 512], F32, name="warm", tag="s")[:, 0:65]
                else:
                    yp = mps.tile([128, 512], F32, name="mp_t", tag="mp")[:, 0:65]
                for tk in range(t + 1):
                    c = OFF[tk] + 128 * (t - tk)
                    nc.tensor.matmul(
                        yp[:],
                        buf[:, c : c + 128],
                        v_sb[tk].rearrange("p (h c) -> p h c", h=4)[:, h],
                        start=(tk == 0),
                        stop=(tk == t),
                        skip_group_check=True,
                    )
                rec = smp.tile([128, 1], F32, name="rec")
                nc.vector.reciprocal(rec[:], yp[:, 64:65])
                nc.vector.tensor_scalar_mul(
                    st[:, 64 * (h % 2) : 64 * (h % 2) + 64], yp[:, 0:64], rec[:]
                )

            def emit_y_pair(hp, t, late=False):
                """y for both heads of the pair into a staging tile."""
                st = smp.tile([128, 128], BF16, name="st")
                for h in (2 * hp, 2 * hp + 1):
                    emit_y_t(h, t, st, late)
                return st

            def emit_yt_transpose(hp, t, st, late=False):
                if late:
                    # tail: PE transpose + copy (shorter latency than the
                    # DMA XBAR path, and psum is free by then)
                    tp = mps.tile([128, 128], BF16, name="mp_t", tag="mp")
                    nc.tensor.transpose(tp[:], st[:], id_sb[:])
                    nc.vector.tensor_copy(
                        yT_all[hp][:, 128 * t : 128 * t + 128], tp[:]
                    )
                else:
                    # mid-stream: XBAR DMA transpose straight into yT — no PE
                    # work, no psum slot, no DVE copy; DMA engines are idle
                    nc.sync.dma_start_transpose(
                        yT_all[hp][:, 128 * t : 128 * t + 128], st[:]
                    )

            def emit_proj_t(t, late):
                pps_t = {}
                for n in range(2):
                    # tail tiles draw psum from the scores pool, which is idle
                    # once the exp stream ends: deeper effective ring
                    if late:
                        pp = sps.tile([128, 1024], F32, name="warm", tag="s")[:, 0:512]
                    else:
                        pp = mps.tile([128, 512], F32, name="mp_t", tag="mp")
                    pps_t[n] = pp
                    nc.tensor.matmul(
                        pp[:],
                        yT_all[0][:, 128 * t : 128 * t + 128],
                        wpT_sb[0][:, 512 * n : 512 * n + 512],
                        start=True,
                        stop=False,
                        skip_group_check=True,
                    )
                ot = outs.tile([128, 1024], BF16, name="ot")
                for n in range(2):
                    pp = pps_t[n]
                    nc.tensor.matmul(
                        pp[:],
                        yT_all[1][:, 128 * t : 128 * t + 128],
                        wpT_sb[1][:, 512 * n : 512 * n + 512],
                        start=False,
                        stop=True,
                        skip_group_check=True,
                    )
                    # tail tiles: ACT is free once the exp stream ends
                    if late and n == 1:
                        nc.scalar.copy(ot[:, 512 * n : 512 * n + 512], pp[:])
                    else:
                        nc.vector.tensor_copy(
                            ot[:, 512 * n : 512 * n + 512], pp[:]
                        )
                eng = nc.gpsimd if t >= 14 else nc.sync
                eng.dma_start(out_d[128 * t : 128 * t + 128, :], ot[:])

            # PE warm-up on the ub tile while input DMAs land
            warm = sps.tile([128, 1024], F32, name="warm", tag="s")
            for i in range(24):
                nc.tensor.matmul(
                    warm[:, 0:128], ub_sb[:], ub_sb[:], start=True, stop=True
                )

            # ---- phase A: heads 0-1 ----
            for tq in range(4):
                emit_qk_group(0, 0, tq)
                emit_qk_group(0, 1, tq)
            # head-pair-1 QKV groups are drip-fed between score tiles of
            # rounds 2-3 so no single PE detour outruns the exp backlog
            qk1_queue = [(1, kind, tq) for tq in range(4) for kind in (0, 1)]
            for g in range(4):
                for h in (0, 1):
                    for tk in range(4 * g, 4 * g + 4):
                        emit_scores_tk(h, tk)
                        if h == 0:
                            emit_v_t(tk)
                        elif g >= 2 and qk1_queue:
                            emit_qk_group(*qk1_queue.pop(0))
                if g > 0:
                    sts = [emit_y_pair(0, t) for t in range(4 * (g - 1), 4 * g)]
                    for i, t in enumerate(range(4 * (g - 1), 4 * g)):
                        emit_yt_transpose(0, t, sts[i])
            sts = [emit_y_pair(0, t) for t in range(12, 16)]
            for i, t in enumerate(range(12, 16)):
                emit_yt_transpose(0, t, sts[i])

            # ---- phase B: heads 2-3 + projection ----
            for g in range(4):
                for h in (2, 3):
                    for tk in range(4 * g, 4 * g + 4):
                        emit_scores_tk(h, tk)
                if g > 0:
                    ts = list(range(4 * (g - 1), 4 * g))
                    sts = [emit_y_pair(1, t) for t in ts]
                    for i, t in enumerate(ts):
                        emit_yt_transpose(1, t, sts[i])
                    for t in ts:
                        emit_proj_t(t, late=False)
            ts = list(range(12, 16))
            sts = [emit_y_pair(1, t, late=True) for t in ts]
            for i, t in enumerate(ts):
                emit_yt_transpose(1, t, sts[i], late=True)
            for t in ts:
                emit_proj_t(t, late=True)

    nc.finalize()
    return nc


def _get_nc():
    global _cached_nc
    if _cached_nc is None:
        _cached_nc = _build()
    return _cached_nc


def kernel(x, Wq, Wk, Wv, Wp, bp):
    global last_results
    x = np.asarray(x, dtype=np.float32)
    Wq = np.asarray(Wq, dtype=np.float32)
    Wk = np.asarray(Wk, dtype=np.float32)
    Wv = np.asarray(Wv, dtype=np.float32)
    Wp = np.asarray(Wp, dtype=np.float32)
    bp = np.asarray(bp, dtype=np.float32)

    WpT = np.ascontiguousarray(Wp.T) / WSCALE  # [C_in(features), C_out]
    # 0/1 causal mask for the tile diagonal block: attT[tk, tq] kept
    # where tq >= tk (upper triangle incl. diagonal)
    ub = np.triu(np.ones((128, 128), dtype=np.float32)).astype(
        ml_dtypes.bfloat16
    )
    id128 = np.eye(128, dtype=np.float32).astype(ml_dtypes.bfloat16)
    f8 = ml_dtypes.float8_e4m3

    def chunked(w):
        # [C, m] -> [128, 8*m]: c-chunk s at cols [m*s : m*(s+1)]
        m = w.shape[1]
        return np.ascontiguousarray(
            w.reshape(8, 128, m).transpose(1, 0, 2).reshape(128, 8 * m)
        )

    def hi_lo(w):
        hi = w.astype(f8)
        lo = (w - hi.astype(np.float32)).astype(f8)
        return hi, lo

    x8_by_b, xl_by_b = [], []
    for b in range(B):
        hi, lo = hi_lo(chunked(np.ascontiguousarray(x[b].T)))
        x8_by_b.append(hi)
        xl_by_b.append(lo)

    in_maps = []
    for core in range(N_CORES):
        b, g = core // 4, core % 4
        h0 = HPC * g
        wq_c = np.stack([chunked(np.concatenate(
            [Wq[h0 + 2 * hp] * WSCALE, Wq[h0 + 2 * hp + 1] * WSCALE], axis=1))
            for hp in range(2)])
        wk_c = np.stack([chunked(np.concatenate(
            [Wk[h0 + 2 * hp] * WSCALE, Wk[h0 + 2 * hp + 1] * WSCALE], axis=1))
            for hp in range(2)])
        wv_c = chunked(np.concatenate(
            [Wv[h0 + j] * WSCALE for j in range(HPC)], axis=1))
        wq_hi, wq_lo = hi_lo(wq_c)
        wk_hi, wk_lo = hi_lo(wk_c)
        wv_hi, wv_lo = hi_lo(wv_c)
        wpT_p = np.ascontiguousarray(
            WpT[256 * g : 256 * (g + 1)].reshape(2, 128, C)
        ).astype(ml_dtypes.bfloat16)
        in_maps.append({
            "xT": x8_by_b[b], "xl": xl_by_b[b],
            "wq": wq_hi, "wql": wq_lo,
            "wk": wk_hi, "wkl": wk_lo,
            "wv": wv_hi, "wvl": wv_lo,
            "wpT": wpT_p, "ub": ub, "id": id128,
        })

    nc = _get_nc()
    kwargs = {}
    if os.environ.get("KERNEL_TRACE", "0") == "1":
        kwargs = dict(trace=True, trace_cores=list(range(N_CORES)),
                      stitch_traces=True)
    try:
        res = run_bass_kernel_spmd(
            nc, in_maps, core_ids=list(range(N_CORES)), **kwargs
        )
    except ModuleNotFoundError:
        res = run_bass_kernel_spmd(nc, in_maps, core_ids=list(range(N_CORES)))
    last_results = res

    out = np.zeros((B, T, C), dtype=np.float32)
    for core in range(N_CORES):
        b = core // 4
        out[b] += res.results[core]["out"].astype(np.float32)
    out += bp[None, None, :]
    return out


# revision 42
# speedup vs baseline: 1.1071x; 1.0180x over previous
"""Multi-head causal attention (B=2, T=2048, C=1024, H=16, S=64) on 8 TRN2 cores.

Sharding: core i handles batch b = i//4 and head group g = i%4 (4 heads each).
Each core computes a partial output projection (its heads' contribution to the
full [T, C] output); the host sums the 4 partials per batch and adds the bias.

Per-core dataflow (f32 PSUM accumulation throughout):
  qT/kT  [128, T] bf16 = (32W).T @ x   -- fp8 DoubleRow over c-chunk pairs,
         3 passes (hi*hi + hi*Wlo + xlo*hi) so fp8 quantization cancels
  v      [128, 4h x 65] bf16 (32-scaled, ones column for the denominator),
         same 3-pass fp8 DR projection
  attT   [tk, tq] psum = kT-tile.T @ qT (bf16, K=64) with a -1e6 upper-
         triangle matmul accumulated onto the diagonal block (causal mask)
  p      = exp(att/(sqrt(S)*1024))     (ACT, bf16 out; mask underflows to 0)
  y      [tq=128, 65] psum per t-tile = p-tile.T @ [v|1]  (flipped: N=65)
  ynorm  = y * recip(d) per-partition (DVE tensor_scalar), staged per
         head-pair then PE-transposed back to yT [s, t] layout
  out    [T, C] partial = yT.T @ (WpT/32) (bf16, head-pair accumulation)

Heads processed in two phases (pair 0-1 then 2-3) so only two bf16 attT
buffers are live; the projection runs in phase 2, gated per t-tile.
"""

import os
import math
import numpy as np
import ml_dtypes

import concourse.bacc as bacc
import concourse.mybir as mybir
import concourse.tile as tile
from concourse.bass_utils import run_bass_kernel_spmd

F32 = mybir.dt.float32
BF16 = mybir.dt.bfloat16
FP8 = mybir.dt.float8e4
DR = mybir.MatmulPerfMode.DoubleRow

B, T, C, H, S = 2, 2048, 1024, 16, 64
HPC = 4          # heads per core
N_CORES = 8
NC_T = T // 128  # 16 t-tiles of 128

WSCALE = 32.0                       # weight scale (fp8 denormal avoidance)
ESCALE = 0.125 / (WSCALE * WSCALE)  # exp scale on (32q)(32k) psum logits
BIGL = 1.0e6                        # causal-mask logit offset (pre-ESCALE)

# attT storage offsets: tile tk spans tq in [128*tk, 2048)
SPAN = [T - 128 * tk for tk in range(NC_T)]
OFF = [0] * NC_T
for _tk in range(1, NC_T):
    OFF[_tk] = OFF[_tk - 1] + SPAN[_tk - 1]
ATT_W = OFF[-1] + SPAN[-1]  # 17408

_cached_nc = None
last_results = None  # BassKernelResults of the most recent run (for test harness)


def _build():
    nc = bacc.Bacc("TRN2", target_bir_lowering=False)

    # contraction row c = 128*s + p lives at partition p, slot s.
    # *l tensors are the fp8 quantization residuals.
    xT_d = nc.dram_tensor("xT", [128, 8 * T], FP8, kind="ExternalInput")
    xl_d = nc.dram_tensor("xl", [128, 8 * T], FP8, kind="ExternalInput")
    wq_d = nc.dram_tensor("wq", [2, 128, 8 * 128], FP8, kind="ExternalInput")
    wk_d = nc.dram_tensor("wk", [2, 128, 8 * 128], FP8, kind="ExternalInput")
    wql_d = nc.dram_tensor("wql", [2, 128, 8 * 128], FP8, kind="ExternalInput")
    wkl_d = nc.dram_tensor("wkl", [2, 128, 8 * 128], FP8, kind="ExternalInput")
    wv_d = nc.dram_tensor("wv", [128, 8 * 256], FP8, kind="ExternalInput")
    wvl_d = nc.dram_tensor("wvl", [128, 8 * 256], FP8, kind="ExternalInput")
    wpT_d = nc.dram_tensor("wpT", [2, 128, C], BF16, kind="ExternalInput")
    ub_d = nc.dram_tensor("ub", [128, 128], BF16, kind="ExternalInput")
    id_d = nc.dram_tensor("id", [128, 128], BF16, kind="ExternalInput")
    out_d = nc.dram_tensor("out", [T, C], BF16, kind="ExternalOutput")

    with tile.TileContext(nc) as tc:
        with (
            tc.tile_pool(name="const", bufs=1) as constp,
            tc.tile_pool(name="qkT", bufs=1) as qkp,
            tc.tile_pool(name="vsb", bufs=1) as vp,
            tc.tile_pool(name="yT", bufs=1) as ytp,
            tc.tile_pool(name="attT", bufs=1) as attp,
            tc.tile_pool(name="sps", bufs=3, space="PSUM") as sps,
            tc.tile_pool(name="mps", bufs=2, space="PSUM") as mps,
            tc.tile_pool(name="xw", bufs=1) as xw,
            tc.tile_pool(name="outs", bufs=4) as outs,
            tc.tile_pool(name="sm", bufs=5) as smp,
        ):
            ub_sb = constp.tile([128, 128], BF16, name="ub_sb")
            id_sb = constp.tile([128, 128], BF16, name="id_sb")
            nc.gpsimd.dma_start(ub_sb[:], ub_d[:])
            nc.gpsimd.dma_start(id_sb[:], id_d[:])
            wpT_sb = [
                constp.tile([128, C], BF16, name=f"wpT{hp}") for hp in range(2)
            ]

            qT2 = [qkp.tile([128, T], BF16, name=f"qT2_{hp}") for hp in range(2)]
            kT2 = [qkp.tile([128, T], BF16, name=f"kT2_{hp}") for hp in range(2)]
            # v tiles: [128, 4 heads x 65] bf16; col 64 of each 65-block = 1
            v_sb = [vp.tile([128, 4 * 65], BF16, name=f"v{t}") for t in range(NC_T)]
            for t in range(NC_T):
                nc.vector.memset(
                    v_sb[t].rearrange("p (h c) -> p h c", h=4)[:, :, 64], 1.0
                )
            yT_all = [ytp.tile([128, T], BF16, name=f"yTa{hp}") for hp in range(2)]
            # three attT buffers: h0->0, h1->1, h2->2, h3->0. The third
            # lets phase B's first head start before phase A's y tail drains.
            ab = [attp.tile([128, ATT_W], BF16, name=f"attb{i}") for i in range(3)]
            ABMAP = {0: 0, 1: 1, 2: 2, 3: 0}

            # input DMAs: x8 pairs + hp0 hi weights first, then residuals
            wq_sb = [xw.tile([128, 1024], FP8, name=f"wq{hp}") for hp in range(2)]
            wk_sb = [xw.tile([128, 1024], FP8, name=f"wk{hp}") for hp in range(2)]
            wql_sb = [xw.tile([128, 1024], FP8, name=f"wql{hp}") for hp in range(2)]
            wkl_sb = [xw.tile([128, 1024], FP8, name=f"wkl{hp}") for hp in range(2)]
            wv_sb = xw.tile([128, 2048], FP8, name="wv")
            wvl_sb = xw.tile([128, 2048], FP8, name="wvl")
            xT_sb = xw.tile([128, 8 * T], FP8, name="xT")
            xl_sb = xw.tile([128, 8 * T], FP8, name="xl")

            nc.sync.dma_start(xT_sb[:, 0:4096], xT_d[:, 0:4096])
            nc.sync.dma_start(wq_sb[0][:], wq_d[0])
            nc.sync.dma_start(wk_sb[0][:], wk_d[0])
            for sp in range(1, 4):
                nc.sync.dma_start(
                    xT_sb[:, 4096 * sp : 4096 * sp + 4096],
                    xT_d[:, 4096 * sp : 4096 * sp + 4096],
                )
            nc.sync.dma_start(wql_sb[0][:], wql_d[0])
            nc.sync.dma_start(wkl_sb[0][:], wkl_d[0])
            for sp in range(4):
                nc.sync.dma_start(
                    xl_sb[:, 4096 * sp : 4096 * sp + 4096],
                    xl_d[:, 4096 * sp : 4096 * sp + 4096],
                )
            nc.sync.dma_start(wv_sb[:], wv_d[:])
            nc.sync.dma_start(wvl_sb[:], wvl_d[:])
            nc.sync.dma_start(wq_sb[1][:], wq_d[1])
            nc.sync.dma_start(wk_sb[1][:], wk_d[1])
            nc.sync.dma_start(wql_sb[1][:], wql_d[1])
            nc.sync.dma_start(wkl_sb[1][:], wkl_d[1])
            for hp in range(2):
                nc.gpsimd.dma_start(wpT_sb[hp][:], wpT_d[hp])

            x_sl = xT_sb.rearrange("p (s t) -> p s t", s=8)
            xl_sl = xl_sb.rearrange("p (s t) -> p s t", s=8)

            def emit_qk_group(hp, kind, tq):
                w_hi = (wq_sb if kind == 0 else wk_sb)[hp]
                w_lo = (wql_sb if kind == 0 else wkl_sb)[hp]
                dst = (qT2 if kind == 0 else kT2)[hp]
                pt = mps.tile([128, 512], F32, name="mp_t", tag="mp")
                passes = ((w_hi, x_sl), (w_lo, x_sl), (w_hi, xl_sl))
                for pi, (wgt, xs) in enumerate(passes):
                    for jj in range(4):
                        lhsT = wgt[:, 256 * jj : 256 * jj + 256].rearrange(
                            "p (j m) -> p j m", j=2
                        )
                        rhs = xs[:, 2 * jj : 2 * jj + 2, 512 * tq : 512 * tq + 512]
                        nc.tensor.matmul(
                            pt[:],
                            lhsT,
                            rhs,
                            start=(pi == 0 and jj == 0),
                            stop=(pi == 2 and jj == 3),
                            perf_mode=DR,
                            skip_group_check=True,
                        )
                # hp0 q copies on ACT (idle pre-stream); rest on DVE
                if kind == 0 and hp == 0:
                    nc.scalar.copy(dst[:, 512 * tq : 512 * tq + 512], pt[:])
                else:
                    nc.vector.tensor_copy(dst[:, 512 * tq : 512 * tq + 512], pt[:])

            def emit_v_t(t):
                pv = mps.tile([128, 512], F32, name="mp_t", tag="mp")
                passes = ((wv_sb, x_sl), (wvl_sb, x_sl), (wv_sb, xl_sl))
                for pi, (wgt, xs) in enumerate(passes):
                    for jj in range(4):
                        lhsT = xs[:, 2 * jj : 2 * jj + 2, 128 * t : 128 * t + 128]
                        rhs = wgt[:, 512 * jj : 512 * jj + 512].rearrange(
                            "p (j m) -> p j m", j=2
                        )
                        nc.tensor.matmul(
                            pv[:, 0:256],
                            lhsT,
                            rhs,
                            start=(pi == 0 and jj == 0),
                            stop=(pi == 2 and jj == 3),
                            perf_mode=DR,
                            skip_group_check=True,
                        )
                dst = v_sb[t].rearrange("p (h c) -> p h c", h=4)[:, :, 0:64]
                nc.vector.tensor_copy(
                    dst, pv[:, 0:256].rearrange("p (h c) -> p h c", h=4)
                )

            def emit_scores_tk(h, tk):
                # h is the core-local head index 0..3; buffer = h % 2
                hp, half = h // 2, h % 2
                r0 = 64 * half
                krow = kT2[hp][r0 : r0 + 64, :]
                qrow = qT2[hp][r0 : r0 + 64, :]
                span = SPAN[tk]
                kt = krow[:, 128 * tk : 128 * tk + 128]
                for part in range(math.ceil(span / 1024)):
                    pspan = min(1024, span - 1024 * part)
                    pt = sps.tile([128, 1024], F32, name="sps_t", tag="s")
                    for mmi in range(math.ceil(pspan / 512)):
                        n = min(512, pspan - 512 * mmi)
                        tq0 = 128 * tk + 1024 * part + 512 * mmi
                        nc.tensor.matmul(
                            pt[:, 512 * mmi : 512 * mmi + n],
                            kt,
                            qrow[:, tq0 : tq0 + n],
                            start=True,
                            stop=True,
                            skip_group_check=True,
                        )

                    dst = ab[ABMAP[h]][
                        :, OFF[tk] + 1024 * part : OFF[tk] + 1024 * part + pspan
                    ]
                    nc.scalar.activation(
                        dst,
                        pt[:, 0:pspan],
                        mybir.ActivationFunctionType.Exp,
                        scale=ESCALE,
                    )
                    if part == 0:
                        diag = ab[ABMAP[h]][:, OFF[tk] : OFF[tk] + 128]
                        nc.vector.tensor_mul(diag, diag, ub_sb[:])

            def emit_y_t(h, t, st, late=False):
                """Flipped y for tq-tile t: yp[tq, s|d] = sum_tk p.T @ [v|1];
                normalize into staging tile st cols [64*(h%2) : +64]."""
                buf = ab[ABMAP[h]][:]
                if late:
                    yp = sps.tile([128, 512], F32, name="warm", tag="s")[:, 0:65]
                else:
                    yp = mps.tile([128, 512], F32, name="mp_t", tag="mp")[:, 0:65]
                for tk in range(t + 1):
                    c = OFF[tk] + 128 * (t - tk)
                    nc.tensor.matmul(
                        yp[:],
                        buf[:, c : c + 128],
                        v_sb[tk].rearrange("p (h c) -> p h c", h=4)[:, h],
                        start=(tk == 0),
                        stop=(tk == t),
                        skip_group_check=True,
                    )
                rec = smp.tile([128, 1], F32, name="rec")
                nc.vector.reciprocal(rec[:], yp[:, 64:65])
                nc.vector.tensor_scalar_mul(
                    st[:, 64 * (h % 2) : 64 * (h % 2) + 64], yp[:, 0:64], rec[:]
                )

            def emit_y_pair(hp, t, late=False):
                """y for both heads of the pair into a staging tile."""
                st = smp.tile([128, 128], BF16, name="st")
                for h in (2 * hp, 2 * hp + 1):
                    emit_y_t(h, t, st, late)
                return st

            def emit_yt_transpose(hp, t, st, late=False):
                if late:
                    # tail: PE transpose + copy (shorter latency than the
                    # DMA XBAR path, and psum is free by then)
                    tp = mps.tile([128, 128], BF16, name="mp_t", tag="mp")
                    nc.tensor.transpose(tp[:], st[:], id_sb[:])
                    nc.vector.tensor_copy(
                        yT_all[hp][:, 128 * t : 128 * t + 128], tp[:]
                    )
                else:
                    # mid-stream: XBAR DMA transpose straight into yT — no PE
                    # work, no psum slot, no DVE copy; DMA engines are idle
                    nc.sync.dma_start_transpose(
                        yT_all[hp][:, 128 * t : 128 * t + 128], st[:]
                    )

            def emit_proj_t(t, late):
                pps_t = {}
                for n in range(2):
                    # tail tiles draw psum from the scores pool, which is idle
                    # once the exp stream ends: deeper effective ring
                    if late:
                        pp = sps.tile([128, 1024], F32, name="warm", tag="s")[:, 0:512]
                    else:
                        pp = mps.tile([128, 512], F32, name="mp_t", tag="mp")
                    pps_t[n] = pp
                    nc.tensor.matmul(
                        pp[:],
                        yT_all[0][:, 128 * t : 128 * t + 128],
                        wpT_sb[0][:, 512 * n : 512 * n + 512],
                        start=True,
                        stop=False,
                        skip_group_check=True,
                    )
                ot = outs.tile([128, 1024], BF16, name="ot")
                for n in range(2):
                    pp = pps_t[n]
                    nc.tensor.matmul(
                        pp[:],
                        yT_all[1][:, 128 * t : 128 * t + 128],
                        wpT_sb[1][:, 512 * n : 512 * n + 512],
                        start=False,
                        stop=True,
                        skip_group_check=True,
                    )
                    # tail tiles: ACT is free once the exp stream ends
                    if late and n == 1:
                        nc.scalar.copy(ot[:, 512 * n : 512 * n + 512], pp[:])
                    else:
                        nc.vector.tensor_copy(
                            ot[:, 512 * n : 512 * n + 512], pp[:]
                        )
                eng = nc.gpsimd if t >= 14 else nc.sync
                eng.dma_start(out_d[128 * t : 128 * t + 128, :], ot[:])

            # PE warm-up on the ub tile while input DMAs land
            warm = sps.tile([128, 1024], F32, name="warm", tag="s")
            for i in range(24):
                nc.tensor.matmul(
                    warm[:, 0:128], ub_sb[:], ub_sb[:], start=True, stop=True
                )

            # ---- phase A: heads 0-1 ----
            for tq in range(4):
                emit_qk_group(0, 0, tq)
                emit_qk_group(0, 1, tq)
            # head-pair-1 QKV groups are drip-fed between score tiles of
            # rounds 2-3 so no single PE detour outruns the exp backlog
            qk1_queue = [(1, kind, tq) for tq in range(4) for kind in (0, 1)]
            for g in range(4):
                for h in (0, 1):
                    for tk in range(4 * g, 4 * g + 4):
                        emit_scores_tk(h, tk)
                        if h == 0:
                            emit_v_t(tk)
                        elif g >= 2 and qk1_queue:
                            emit_qk_group(*qk1_queue.pop(0))
                if g > 0:
                    sts = [emit_y_pair(0, t) for t in range(4 * (g - 1), 4 * g)]
                    for i, t in enumerate(range(4 * (g - 1), 4 * g)):
                        emit_yt_transpose(0, t, sts[i])
            sts = [emit_y_pair(0, t) for t in range(12, 16)]
            for i, t in enumerate(range(12, 16)):
                emit_yt_transpose(0, t, sts[i])

            # ---- phase B: heads 2-3 + projection ----
            for g in range(4):
                for h in (2, 3):
                    for tk in range(4 * g, 4 * g + 4):
                        emit_scores_tk(h, tk)
                if g > 0:
                    ts = list(range(4 * (g - 1), 4 * g))
                    sts = [emit_y_pair(1, t) for t in ts]
                    for i, t in enumerate(ts):
                        emit_yt_transpose(1, t, sts[i])
                    for t in ts:
                        emit_proj_t(t, late=False)
            ts = list(range(12, 16))
            sts = [emit_y_pair(1, t, late=True) for t in ts]
            for i, t in enumerate(ts):
                emit_yt_transpose(1, t, sts[i], late=True)
            for t in ts:
                emit_proj_t(t, late=True)

    nc.finalize()
    return nc


def _get_nc():
    global _cached_nc
    if _cached_nc is None:
        _cached_nc = _build()
    return _cached_nc


def kernel(x, Wq, Wk, Wv, Wp, bp):
    global last_results
    x = np.asarray(x, dtype=np.float32)
    Wq = np.asarray(Wq, dtype=np.float32)
    Wk = np.asarray(Wk, dtype=np.float32)
    Wv = np.asarray(Wv, dtype=np.float32)
    Wp = np.asarray(Wp, dtype=np.float32)
    bp = np.asarray(bp, dtype=np.float32)

    WpT = np.ascontiguousarray(Wp.T) / WSCALE  # [C_in(features), C_out]
    # 0/1 causal mask for the tile diagonal block: attT[tk, tq] kept
    # where tq >= tk (upper triangle incl. diagonal)
    ub = np.triu(np.ones((128, 128), dtype=np.float32)).astype(
        ml_dtypes.bfloat16
    )
    id128 = np.eye(128, dtype=np.float32).astype(ml_dtypes.bfloat16)
    f8 = ml_dtypes.float8_e4m3

    def chunked(w):
        # [C, m] -> [128, 8*m]: c-chunk s at cols [m*s : m*(s+1)]
        m = w.shape[1]
        return np.ascontiguousarray(
            w.reshape(8, 128, m).transpose(1, 0, 2).reshape(128, 8 * m)
        )

    def hi_lo(w):
        hi = w.astype(f8)
        lo = (w - hi.astype(np.float32)).astype(f8)
        return hi, lo

    x8_by_b, xl_by_b = [], []
    for b in range(B):
        hi, lo = hi_lo(chunked(np.ascontiguousarray(x[b].T)))
        x8_by_b.append(hi)
        xl_by_b.append(lo)

    in_maps = []
    for core in range(N_CORES):
        b, g = core // 4, core % 4
        h0 = HPC * g
        wq_c = np.stack([chunked(np.concatenate(
            [Wq[h0 + 2 * hp] * WSCALE, Wq[h0 + 2 * hp + 1] * WSCALE], axis=1))
            for hp in range(2)])
        wk_c = np.stack([chunked(np.concatenate(
            [Wk[h0 + 2 * hp] * WSCALE, Wk[h0 + 2 * hp + 1] * WSCALE], axis=1))
            for hp in range(2)])
        wv_c = chunked(np.concatenate(
            [Wv[h0 + j] * WSCALE for j in range(HPC)], axis=1))
        wq_hi, wq_lo = hi_lo(wq_c)
        wk_hi, wk_lo = hi_lo(wk_c)
        wv_hi, wv_lo = hi_lo(wv_c)
        wpT_p = np.ascontiguousarray(
            WpT[256 * g : 256 * (g + 1)].reshape(2, 128, C)
        ).astype(ml_dtypes.bfloat16)
        in_maps.append({
            "xT": x8_by_b[b], "xl": xl_by_b[b],
            "wq": wq_hi, "wql": wq_lo,
            "wk": wk_hi, "wkl": wk_lo,
            "wv": wv_hi, "wvl": wv_lo,
            "wpT": wpT_p, "ub": ub, "id": id128,
        })

    nc = _get_nc()
    kwargs = {}
    if os.environ.get("KERNEL_TRACE", "0") == "1":
        kwargs = dict(trace=True, trace_cores=list(range(N_CORES)),
                      stitch_traces=True)
    try:
        res = run_bass_kernel_spmd(
            nc, in_maps, core_ids=list(range(N_CORES)), **kwargs
        )
    except ModuleNotFoundError:
        res = run_bass_kernel_spmd(nc, in_maps, core_ids=list(range(N_CORES)))
    last_results = res

    out = np.zeros((B, T, C), dtype=np.float32)
    for core in range(N_CORES):
        b = core // 4
        out[b] += res.results[core]["out"].astype(np.float32)
    out += bp[None, None, :]
    return out


# revision 45
# speedup vs baseline: 1.1115x; 1.0040x over previous
"""Multi-head causal attention (B=2, T=2048, C=1024, H=16, S=64) on 8 TRN2 cores.

Sharding: core i handles batch b = i//4 and head group g = i%4 (4 heads each).
Each core computes a partial output projection (its heads' contribution to the
full [T, C] output); the host sums the 4 partials per batch and adds the bias.

Per-core dataflow (f32 PSUM accumulation throughout):
  qT/kT  [128, T] bf16 = (32W).T @ x   -- fp8 DoubleRow over c-chunk pairs,
         3 passes (hi*hi + hi*Wlo + xlo*hi) so fp8 quantization cancels
  v      [128, 4h x 65] bf16 (32-scaled, ones column for the denominator),
         same 3-pass fp8 DR projection
  attT   [tk, tq] psum = kT-tile.T @ qT (bf16, K=64) with a -1e6 upper-
         triangle matmul accumulated onto the diagonal block (causal mask)
  p      = exp(att/(sqrt(S)*1024))     (ACT, bf16 out; mask underflows to 0)
  y      [tq=128, 65] psum per t-tile = p-tile.T @ [v|1]  (flipped: N=65)
  ynorm  = y * recip(d) per-partition (DVE tensor_scalar), staged per
         head-pair then PE-transposed back to yT [s, t] layout
  out    [T, C] partial = yT.T @ (WpT/32) (bf16, head-pair accumulation)

Heads processed in two phases (pair 0-1 then 2-3) so only two bf16 attT
buffers are live; the projection runs in phase 2, gated per t-tile.
"""

import os
import math
import numpy as np
import ml_dtypes

import concourse.bacc as bacc
import concourse.mybir as mybir
import concourse.tile as tile
from concourse.bass_utils import run_bass_kernel_spmd

F32 = mybir.dt.float32
BF16 = mybir.dt.bfloat16
FP8 = mybir.dt.float8e4
DR = mybir.MatmulPerfMode.DoubleRow

B, T, C, H, S = 2, 2048, 1024, 16, 64
HPC = 4          # heads per core
N_CORES = 8
NC_T = T // 128  # 16 t-tiles of 128

WSCALE = 32.0                       # weight scale (fp8 denormal avoidance)
ESCALE = 0.125 / (WSCALE * WSCALE)  # exp scale on (32q)(32k) psum logits
BIGL = 1.0e6                        # causal-mask logit offset (pre-ESCALE)

# attT storage offsets: tile tk spans tq in [128*tk, 2048)
SPAN = [T - 128 * tk for tk in range(NC_T)]
OFF = [0] * NC_T
for _tk in range(1, NC_T):
    OFF[_tk] = OFF[_tk - 1] + SPAN[_tk - 1]
ATT_W = OFF[-1] + SPAN[-1]  # 17408

_cached_nc = None
last_results = None  # BassKernelResults of the most recent run (for test harness)


def _build():
    nc = bacc.Bacc("TRN2", target_bir_lowering=False)

    # contraction row c = 128*s + p lives at partition p, slot s.
    # *l tensors are the fp8 quantization residuals.
    xT_d = nc.dram_tensor("xT", [128, 8 * T], FP8, kind="ExternalInput")
    xl_d = nc.dram_tensor("xl", [128, 8 * T], FP8, kind="ExternalInput")
    wq_d = nc.dram_tensor("wq", [2, 128, 8 * 128], FP8, kind="ExternalInput")
    wk_d = nc.dram_tensor("wk", [2, 128, 8 * 128], FP8, kind="ExternalInput")
    wql_d = nc.dram_tensor("wql", [2, 128, 8 * 128], FP8, kind="ExternalInput")
    wkl_d = nc.dram_tensor("wkl", [2, 128, 8 * 128], FP8, kind="ExternalInput")
    wv_d = nc.dram_tensor("wv", [128, 8 * 256], FP8, kind="ExternalInput")
    wvl_d = nc.dram_tensor("wvl", [128, 8 * 256], FP8, kind="ExternalInput")
    wpT_d = nc.dram_tensor("wpT", [2, 128, C], BF16, kind="ExternalInput")
    ub_d = nc.dram_tensor("ub", [128, 128], BF16, kind="ExternalInput")
    id_d = nc.dram_tensor("id", [128, 128], BF16, kind="ExternalInput")
    out_d = nc.dram_tensor("out", [T, C], BF16, kind="ExternalOutput")

    with tile.TileContext(nc) as tc:
        with (
            tc.tile_pool(name="const", bufs=1) as constp,
            tc.tile_pool(name="qkT", bufs=1) as qkp,
            tc.tile_pool(name="vsb", bufs=1) as vp,
            tc.tile_pool(name="yT", bufs=1) as ytp,
            tc.tile_pool(name="attT", bufs=1) as attp,
            tc.tile_pool(name="sps", bufs=3, space="PSUM") as sps,
            tc.tile_pool(name="mps", bufs=2, space="PSUM") as mps,
            tc.tile_pool(name="xw", bufs=1) as xw,
            tc.tile_pool(name="outs", bufs=4) as outs,
            tc.tile_pool(name="sm", bufs=5) as smp,
        ):
            ub_sb = constp.tile([128, 128], BF16, name="ub_sb")
            id_sb = constp.tile([128, 128], BF16, name="id_sb")
            nc.gpsimd.dma_start(ub_sb[:], ub_d[:])
            nc.gpsimd.dma_start(id_sb[:], id_d[:])
            wpT_sb = [
                constp.tile([128, C], BF16, name=f"wpT{hp}") for hp in range(2)
            ]

            qT2 = [qkp.tile([128, T], BF16, name=f"qT2_{hp}") for hp in range(2)]
            kT2 = [qkp.tile([128, T], BF16, name=f"kT2_{hp}") for hp in range(2)]
            # v tiles: [128, 4 heads x 65] bf16; col 64 of each 65-block = 1
            v_sb = [vp.tile([128, 4 * 65], BF16, name=f"v{t}") for t in range(NC_T)]
            for t in range(NC_T):
                nc.vector.memset(
                    v_sb[t].rearrange("p (h c) -> p h c", h=4)[:, :, 64], 1.0
                )
            yT_all = [ytp.tile([128, T], BF16, name=f"yTa{hp}") for hp in range(2)]
            # three attT buffers: h0->0, h1->1, h2->2, h3->0. The third
            # lets phase B's first head start before phase A's y tail drains.
            ab = [attp.tile([128, ATT_W], BF16, name=f"attb{i}") for i in range(3)]
            ABMAP = {0: 0, 1: 1, 2: 2, 3: 0}

            # input DMAs: x8 pairs + hp0 hi weights first, then residuals
            wq_sb = [xw.tile([128, 1024], FP8, name=f"wq{hp}") for hp in range(2)]
            wk_sb = [xw.tile([128, 1024], FP8, name=f"wk{hp}") for hp in range(2)]
            wql_sb = [xw.tile([128, 1024], FP8, name=f"wql{hp}") for hp in range(2)]
            wkl_sb = [xw.tile([128, 1024], FP8, name=f"wkl{hp}") for hp in range(2)]
            wv_sb = xw.tile([128, 2048], FP8, name="wv")
            wvl_sb = xw.tile([128, 2048], FP8, name="wvl")
            xT_sb = xw.tile([128, 8 * T], FP8, name="xT")
            xl_sb = xw.tile([128, 8 * T], FP8, name="xl")

            nc.sync.dma_start(xT_sb[:, 0:4096], xT_d[:, 0:4096])
            nc.sync.dma_start(wq_sb[0][:], wq_d[0])
            nc.sync.dma_start(wk_sb[0][:], wk_d[0])
            for sp in range(1, 4):
                nc.sync.dma_start(
                    xT_sb[:, 4096 * sp : 4096 * sp + 4096],
                    xT_d[:, 4096 * sp : 4096 * sp + 4096],
                )
            nc.sync.dma_start(wql_sb[0][:], wql_d[0])
            nc.sync.dma_start(wkl_sb[0][:], wkl_d[0])
            for sp in range(4):
                nc.sync.dma_start(
                    xl_sb[:, 4096 * sp : 4096 * sp + 4096],
                    xl_d[:, 4096 * sp : 4096 * sp + 4096],
                )
            nc.sync.dma_start(wv_sb[:], wv_d[:])
            nc.sync.dma_start(wvl_sb[:], wvl_d[:])
            nc.sync.dma_start(wq_sb[1][:], wq_d[1])
            nc.sync.dma_start(wk_sb[1][:], wk_d[1])
            nc.sync.dma_start(wql_sb[1][:], wql_d[1])
            nc.sync.dma_start(wkl_sb[1][:], wkl_d[1])
            for hp in range(2):
                nc.gpsimd.dma_start(wpT_sb[hp][:], wpT_d[hp])

            x_sl = xT_sb.rearrange("p (s t) -> p s t", s=8)
            xl_sl = xl_sb.rearrange("p (s t) -> p s t", s=8)

            def emit_qk_group(hp, kind, tq):
                w_hi = (wq_sb if kind == 0 else wk_sb)[hp]
                w_lo = (wql_sb if kind == 0 else wkl_sb)[hp]
                dst = (qT2 if kind == 0 else kT2)[hp]
                pt = mps.tile([128, 512], F32, name="mp_t", tag="mp")
                passes = ((w_hi, x_sl), (w_lo, x_sl), (w_hi, xl_sl))
                for pi, (wgt, xs) in enumerate(passes):
                    for jj in range(4):
                        lhsT = wgt[:, 256 * jj : 256 * jj + 256].rearrange(
                            "p (j m) -> p j m", j=2
                        )
                        rhs = xs[:, 2 * jj : 2 * jj + 2, 512 * tq : 512 * tq + 512]
                        nc.tensor.matmul(
                            pt[:],
                            lhsT,
                            rhs,
                            start=(pi == 0 and jj == 0),
                            stop=(pi == 2 and jj == 3),
                            perf_mode=DR,
                            skip_group_check=True,
                        )
                # hp0 q copies on ACT (idle pre-stream); rest on DVE
                if kind == 0 and hp == 0:
                    nc.scalar.copy(dst[:, 512 * tq : 512 * tq + 512], pt[:])
                else:
                    nc.vector.tensor_copy(dst[:, 512 * tq : 512 * tq + 512], pt[:])

            def emit_v_t(t):
                pv = mps.tile([128, 512], F32, name="mp_t", tag="mp")
                passes = ((wv_sb, x_sl), (wvl_sb, x_sl), (wv_sb, xl_sl))
                for pi, (wgt, xs) in enumerate(passes):
                    for jj in range(4):
                        lhsT = xs[:, 2 * jj : 2 * jj + 2, 128 * t : 128 * t + 128]
                        rhs = wgt[:, 512 * jj : 512 * jj + 512].rearrange(
                            "p (j m) -> p j m", j=2
                        )
                        nc.tensor.matmul(
                            pv[:, 0:256],
                            lhsT,
                            rhs,
                            start=(pi == 0 and jj == 0),
                            stop=(pi == 2 and jj == 3),
                            perf_mode=DR,
                            skip_group_check=True,
                        )
                dst = v_sb[t].rearrange("p (h c) -> p h c", h=4)[:, :, 0:64]
                nc.vector.tensor_copy(
                    dst, pv[:, 0:256].rearrange("p (h c) -> p h c", h=4)
                )

            def emit_scores_tk(h, tk, parts=None):
                # h is the core-local head index 0..3; buffer = h % 2
                hp, half = h // 2, h % 2
                r0 = 64 * half
                krow = kT2[hp][r0 : r0 + 64, :]
                qrow = qT2[hp][r0 : r0 + 64, :]
                span = SPAN[tk]
                kt = krow[:, 128 * tk : 128 * tk + 128]
                for part in (range(math.ceil(span / 1024))
                             if parts is None else parts):
                    pspan = min(1024, span - 1024 * part)
                    pt = sps.tile([128, 1024], F32, name="sps_t", tag="s")
                    for mmi in range(math.ceil(pspan / 512)):
                        n = min(512, pspan - 512 * mmi)
                        tq0 = 128 * tk + 1024 * part + 512 * mmi
                        nc.tensor.matmul(
                            pt[:, 512 * mmi : 512 * mmi + n],
                            kt,
                            qrow[:, tq0 : tq0 + n],
                            start=True,
                            stop=True,
                            skip_group_check=True,
                        )

                    dst = ab[ABMAP[h]][
                        :, OFF[tk] + 1024 * part : OFF[tk] + 1024 * part + pspan
                    ]
                    nc.scalar.activation(
                        dst,
                        pt[:, 0:pspan],
                        mybir.ActivationFunctionType.Exp,
                        scale=ESCALE,
                    )
                    if part == 0:
                        diag = ab[ABMAP[h]][:, OFF[tk] : OFF[tk] + 128]
                        nc.vector.tensor_mul(diag, diag, ub_sb[:])

            def emit_y_t(h, t, st, late=False):
                """Flipped y for tq-tile t: yp[tq, s|d] = sum_tk p.T @ [v|1];
                normalize into staging tile st cols [64*(h%2) : +64]."""
                buf = ab[ABMAP[h]][:]
                if late:
                    yp = sps.tile([128, 512], F32, name="warm", tag="s")[:, 0:65]
                else:
                    yp = mps.tile([128, 512], F32, name="mp_t", tag="mp")[:, 0:65]
                for tk in range(t + 1):
                    c = OFF[tk] + 128 * (t - tk)
                    nc.tensor.matmul(
                        yp[:],
                        buf[:, c : c + 128],
                        v_sb[tk].rearrange("p (h c) -> p h c", h=4)[:, h],
                        start=(tk == 0),
                        stop=(tk == t),
                        skip_group_check=True,
                    )
                rec = smp.tile([128, 1], F32, name="rec")
                nc.vector.reciprocal(rec[:], yp[:, 64:65])
                nc.vector.tensor_scalar_mul(
                    st[:, 64 * (h % 2) : 64 * (h % 2) + 64], yp[:, 0:64], rec[:]
                )

            def emit_y_pair(hp, t, late=False):
                """y for both heads of the pair into a staging tile."""
                st = smp.tile([128, 128], BF16, name="st")
                for h in (2 * hp, 2 * hp + 1):
                    emit_y_t(h, t, st, late)
                return st

            def emit_yt_transpose(hp, t, st, late=False):
                if late:
                    # tail: PE transpose + copy (shorter latency than the
                    # DMA XBAR path, and psum is free by then)
                    tp = mps.tile([128, 128], BF16, name="mp_t", tag="mp")
                    nc.tensor.transpose(tp[:], st[:], id_sb[:])
                    nc.vector.tensor_copy(
                        yT_all[hp][:, 128 * t : 128 * t + 128], tp[:]
                    )
                else:
                    # mid-stream: XBAR DMA transpose straight into yT — no PE
                    # work, no psum slot, no DVE copy; DMA engines are idle
                    nc.sync.dma_start_transpose(
                        yT_all[hp][:, 128 * t : 128 * t + 128], st[:]
                    )

            def emit_proj_t(t, late):
                pps_t = {}
                for n in range(2):
                    # tail tiles draw psum from the scores pool, which is idle
                    # once the exp stream ends: deeper effective ring
                    if late:
                        pp = sps.tile([128, 1024], F32, name="warm", tag="s")[:, 0:512]
                    else:
                        pp = mps.tile([128, 512], F32, name="mp_t", tag="mp")
                    pps_t[n] = pp
                    nc.tensor.matmul(
                        pp[:],
                        yT_all[0][:, 128 * t : 128 * t + 128],
                        wpT_sb[0][:, 512 * n : 512 * n + 512],
                        start=True,
                        stop=False,
                        skip_group_check=True,
                    )
                ot = outs.tile([128, 1024], BF16, name="ot")
                for n in range(2):
                    pp = pps_t[n]
                    nc.tensor.matmul(
                        pp[:],
                        yT_all[1][:, 128 * t : 128 * t + 128],
                        wpT_sb[1][:, 512 * n : 512 * n + 512],
                        start=False,
                        stop=True,
                        skip_group_check=True,
                    )
                    # tail tiles: ACT is free once the exp stream ends
                    if late and n == 1:
                        nc.scalar.copy(ot[:, 512 * n : 512 * n + 512], pp[:])
                    else:
                        nc.vector.tensor_copy(
                            ot[:, 512 * n : 512 * n + 512], pp[:]
                        )
                eng = nc.gpsimd if t >= 14 else nc.sync
                eng.dma_start(out_d[128 * t : 128 * t + 128, :], ot[:])

            # PE warm-up on the ub tile while input DMAs land
            warm = sps.tile([128, 1024], F32, name="warm", tag="s")
            for i in range(24):
                nc.tensor.matmul(
                    warm[:, 0:128], ub_sb[:], ub_sb[:], start=True, stop=True
                )

            # ---- phase A: heads 0-1 ----
            # first score part emitted after only 2 q-groups so the exp
            # stream starts as early as possible
            for tq in range(2):
                emit_qk_group(0, 0, tq)
                emit_qk_group(0, 1, tq)
            emit_scores_tk(0, 0, parts=[0])
            for tq in range(2, 4):
                emit_qk_group(0, 0, tq)
                emit_qk_group(0, 1, tq)
            emit_scores_tk(0, 0, parts=[1])
            # head-pair-1 QKV groups are drip-fed between score tiles of
            # rounds 2-3 so no single PE detour outruns the exp backlog
            qk1_queue = [(1, kind, tq) for tq in range(4) for kind in (0, 1)]
            for g in range(4):
                for h in (0, 1):
                    for tk in range(4 * g, 4 * g + 4):
                        if g == 0 and h == 0 and tk == 0:
                            continue
                        emit_scores_tk(h, tk)
                        if h == 0:
                            emit_v_t(tk)
                        elif g >= 2 and qk1_queue:
                            emit_qk_group(*qk1_queue.pop(0))
                if g > 0:
                    sts = [emit_y_pair(0, t) for t in range(4 * (g - 1), 4 * g)]
                    for i, t in enumerate(range(4 * (g - 1), 4 * g)):
                        emit_yt_transpose(0, t, sts[i])
            sts = [emit_y_pair(0, t) for t in range(12, 16)]
            for i, t in enumerate(range(12, 16)):
                emit_yt_transpose(0, t, sts[i])

            # ---- phase B: heads 2-3 + projection ----
            for g in range(4):
                for h in (2, 3):
                    for tk in range(4 * g, 4 * g + 4):
                        emit_scores_tk(h, tk)
                if g > 0:
                    ts = list(range(4 * (g - 1), 4 * g))
                    sts = [emit_y_pair(1, t) for t in ts]
                    for i, t in enumerate(ts):
                        emit_yt_transpose(1, t, sts[i])
                    for t in ts:
                        emit_proj_t(t, late=False)
            ts = list(range(12, 16))
            sts = [emit_y_pair(1, t, late=True) for t in ts]
            for i, t in enumerate(ts):
                emit_yt_transpose(1, t, sts[i], late=True)
            for t in ts:
                emit_proj_t(t, late=True)

    nc.finalize()
    return nc


def _get_nc():
    global _cached_nc
    if _cached_nc is None:
        _cached_nc = _build()
    return _cached_nc


def kernel(x, Wq, Wk, Wv, Wp, bp):
    global last_results
    x = np.asarray(x, dtype=np.float32)
    Wq = np.asarray(Wq, dtype=np.float32)
    Wk = np.asarray(Wk, dtype=np.float32)
    Wv = np.asarray(Wv, dtype=np.float32)
    Wp = np.asarray(Wp, dtype=np.float32)
    bp = np.asarray(bp, dtype=np.float32)

    WpT = np.ascontiguousarray(Wp.T) / WSCALE  # [C_in(features), C_out]
    # 0/1 causal mask for the tile diagonal block: attT[tk, tq] kept
    # where tq >= tk (upper triangle incl. diagonal)
    ub = np.triu(np.ones((128, 128), dtype=np.float32)).astype(
        ml_dtypes.bfloat16
    )
    id128 = np.eye(128, dtype=np.float32).astype(ml_dtypes.bfloat16)
    f8 = ml_dtypes.float8_e4m3

    def chunked(w):
        # [C, m] -> [128, 8*m]: c-chunk s at cols [m*s : m*(s+1)]
        m = w.shape[1]
        return np.ascontiguousarray(
            w.reshape(8, 128, m).transpose(1, 0, 2).reshape(128, 8 * m)
        )

    def hi_lo(w):
        hi = w.astype(f8)
        lo = (w - hi.astype(np.float32)).astype(f8)
        return hi, lo

    x8_by_b, xl_by_b = [], []
    for b in range(B):
        hi, lo = hi_lo(chunked(np.ascontiguousarray(x[b].T)))
        x8_by_b.append(hi)
        xl_by_b.append(lo)

    in_maps = []
    for core in range(N_CORES):
        b, g = core // 4, core % 4
        h0 = HPC * g
        wq_c = np.stack([chunked(np.concatenate(
            [Wq[h0 + 2 * hp] * WSCALE, Wq[h0 + 2 * hp + 1] * WSCALE], axis=1))
            for hp in range(2)])
        wk_c = np.stack([chunked(np.concatenate(
            [Wk[h0 + 2 * hp] * WSCALE, Wk[h0 + 2 * hp + 1] * WSCALE], axis=1))
            for hp in range(2)])
        wv_c = chunked(np.concatenate(
            [Wv[h0 + j] * WSCALE for j in range(HPC)], axis=1))
        wq_hi, wq_lo = hi_lo(wq_c)
        wk_hi, wk_lo = hi_lo(wk_c)
        wv_hi, wv_lo = hi_lo(wv_c)
        wpT_p = np.ascontiguousarray(
            WpT[256 * g : 256 * (g + 1)].reshape(2, 128, C)
        ).astype(ml_dtypes.bfloat16)
        in_maps.append({
            "xT": x8_by_b[b], "xl": xl_by_b[b],
            "wq": wq_hi, "wql": wq_lo,
            "wk": wk_hi, "wkl": wk_lo,
            "wv": wv_hi, "wvl": wv_lo,
            "wpT": wpT_p, "ub": ub, "id": id128,
        })

    nc = _get_nc()
    kwargs = {}
    if os.environ.get("KERNEL_TRACE", "0") == "1":
        kwargs = dict(trace=True, trace_cores=list(range(N_CORES)),
                      stitch_traces=True)
    try:
        res = run_bass_kernel_spmd(
            nc, in_maps, core_ids=list(range(N_CORES)), **kwargs
        )
    except ModuleNotFoundError:
        res = run_bass_kernel_spmd(nc, in_maps, core_ids=list(range(N_CORES)))
    last_results = res

    out = np.zeros((B, T, C), dtype=np.float32)
    for core in range(N_CORES):
        b = core // 4
        out[b] += res.results[core]["out"].astype(np.float32)
    out += bp[None, None, :]
    return out
